# revision 26
# baseline (speedup 1.0000x reference)
"""GRACE contrastive loss kernel for Trainium2 (8 NeuronCores, SPMD).

Strategy (row-block data parallel + symmetry harvesting):
  - Shard the N=8192 nodes across 8 cores (NB=1024 rows each).  Each core
    projects its z1/z2 block through the 2-layer MLP (fp16 matmuls, fp32
    accum), row-normalizes (DVE bit-trick rsqrt, no act-table switch), and
    AllGathers the normalized embeddings (fp16).
  - Similarity work per core is 18 block-units of [1024 x 1024] x K=512
    (vs 32 for the naive 4-matrix scheme):
      * S12 (between_sim): all 8 column blocks — row sums via exp accum_out;
        column sums (ones-matmuls over the exp tiles) give the row sums of
        S21 = S12^T, so S21 is never materialized.
      * S11/S22 (refl_sims): diagonal block locally (no gather needed, runs
        under the AllGathers), plus a shift-invariant triangular assignment:
        unit u in {1,2,3} -> S11 block (c+u)%8, u in {5,6,7} -> S22 block
        (c+u)%8, and u=4 computed by BOTH pair members for BOTH matrices
        with exp pre-halved via bias=-ln(2).  Column sums of each
        off-diagonal exp tile are the transposed block's row-sum
        contribution.
  - All cross-core terms meet in one ReduceScatter over [slot(8) x class(3)
    x 1024] f32; slot c returns exactly core c's total S11/S22/S21 row sums.
    Core-dependent gather/scatter positions use dynamic (register) offsets
    read from tiny per-core uint32 index tables.
"""

import math
import sys

import numpy as np

sys.path.insert(0, "/opt/trn_rl_repo")

import concourse.bass as bass  # noqa: E402
import concourse.mybir as mybir  # noqa: E402
import concourse.tile as tile  # noqa: E402
from concourse import bacc  # noqa: E402
from concourse.bass_utils import run_bass_kernel_spmd  # noqa: E402

F32 = mybir.dt.float32
F16 = mybir.dt.float16
F8 = mybir.dt.float8e4
U32 = mybir.dt.uint32
DR = mybir.MatmulPerfMode.DoubleRow
AF = mybir.ActivationFunctionType
ALU = mybir.AluOpType
SP = mybir.EngineType.SP
PE = mybir.EngineType.PE

N_CORES = 8
N = 8192
D = 512            # feature dim (= H = P in the reference MLP)
NB = N // N_CORES  # 1024 rows per core
KT = D // 128      # 4 k-subtiles
KT2 = KT // 2      # 2 DoubleRow k-subtiles (K=256 each)
F8SCALE = 16.0     # embeddings are shipped as fp8e4 * F8SCALE
MT = NB // 128     # 8 row tiles per core
NCHUNK = 512       # matmul moving free dim (one PSUM bank)
TAU_INV = 2.0      # 1 / tau
E2 = float(np.exp(2.0, dtype=np.float64))  # exp(diag(refl_sim)/tau)
LN2 = float(np.log(2.0))
RSQRT_MAGIC = 0x5F3759DF

TRACE = False
DEBUG = False
LAST_EXEC_NS = None
_CACHE = {}


def _build_program():
    nc = bacc.Bacc("TRN2", target_bir_lowering=False, debug=False,
                   num_devices=N_CORES)

    # ---- I/O ----
    zt1 = nc.dram_tensor("zt1", [128, KT, NB], F16, kind="ExternalInput").ap()
    zt2 = nc.dram_tensor("zt2", [128, KT, NB], F16, kind="ExternalInput").ap()
    w1t = nc.dram_tensor("w1t", [128, KT, D], F16, kind="ExternalInput").ap()
    w2t = nc.dram_tensor("w2t", [128, KT, D], F16, kind="ExternalInput").ap()
    b1 = nc.dram_tensor("b1", [128, KT], F32, kind="ExternalInput").ap()
    b2 = nc.dram_tensor("b2", [128, KT], F32, kind="ExternalInput").ap()
    # per-core index tables (uint32):
    #   gidx[0, u] u=1..4 : gather block (c+u)%8 in the n1 half
    #   gidx[1, u] u=4..7 : gather block (c+u)%8 in the n2 half
    #   gidx[2, 0]        : my slot c
    gidx = nc.dram_tensor("gidx", [3, 8], U32, kind="ExternalInput").ap()
    ident = nc.dram_tensor("ident", [128, 128], F32, kind="ExternalInput").ap()
    out = nc.dram_tensor("out", [1, 1], F32, kind="ExternalOutput").ap()
    if DEBUG:
        dbg_rn = nc.dram_tensor("dbg_rn", [2, NB], F32,
                                kind="ExternalOutput").ap()
        dbg_rs = nc.dram_tensor("dbg_rs", [128, 3 * MT], F32,
                                kind="ExternalOutput").ap()
        dbg_d = nc.dram_tensor("dbg_d", [128, 4 * MT], F32,
                               kind="ExternalOutput").ap()

    rg = [list(range(N_CORES))]

    with tile.TileContext(nc) as tc:
        with tc.tile_pool(name="persist", bufs=1) as persist, \
             tc.tile_pool(name="dram", bufs=1, space="DRAM") as dram:

            ones_col = persist.tile([128, 1], F32)
            nc.vector.memset(ones_col[:], 1.0)
            ones_h = persist.tile([128, 1], F16)
            nc.vector.memset(ones_h[:], 1.0)
            ones_row = persist.tile([1, 128], F32)
            nc.vector.memset(ones_row[:], F8SCALE)
            negln2 = persist.tile([128, 1], F32)
            nc.vector.memset(negln2[:], -LN2)
            id_sb = persist.tile([128, 128], F32, name="id_sb")
            nc.sync.dma_start(id_sb[:], ident)

            # local normalized blocks [feature, node] fp8 * F8SCALE;
            # DoubleRow reads kt-pairs as strided [Ki, Ko=2, *] planes
            n_loc = [persist.tile([128, KT, NB], F8, name=f"n{v}_loc")
                     for v in range(2)]
            rn_vec = [persist.tile([1, NB], F32, name=f"rn{v}") for v in range(2)]

            # ship layout = the SBUF n_loc layout, flattened: [128, 4096] f8
            cc_in = [dram.tile([128, KT * NB], F8, name=f"cc_in{v}")
                     for v in range(2)]
            cc_out = [dram.tile([N_CORES * 128, KT * NB], F8,
                                name=f"cc_out{v}", addr_space="Shared",
                                tag=("agbuf0" if v == 0 else "agbuf1"))
                      for v in range(2)]

            # ReduceScatter staging [slot, class, row] f32;
            # class 0: S11 rowsums, 1: S22 rowsums, 2: S21 rowsums (=colsum S12)
            rs_in = dram.tile([N_CORES, 3, NB], F32, name="rs_in")
            rs_out = dram.tile([3, NB], F32, name="rs_out")

            # rowsum partials from exp accum_out
            parts12 = persist.tile([128, MT, 8], F32, name="parts12")
            parts11 = persist.tile([128, MT, 5], F32, name="parts11")
            parts22 = persist.tile([128, MT, 5], F32, name="parts22")
            pos_sum = persist.tile([1, 1], F32)

            # ---- dynamic index registers ----
            # engines can't read DRAM and the runtime bounds-check assert is
            # broken under this runtime: stage the table in SBUF and skip
            # the runtime check (bounds are guaranteed host-side).
            gidx_sb = persist.tile([1, 24], U32, name="gidx_sb")
            nc.sync.dma_start(gidx_sb[:],
                              gidx[:].rearrange("a b -> (a b)").unsqueeze(0))

            def ld(flat, engines):
                return nc.values_load(gidx_sb[0:1, flat:flat + 1],
                                      engines=engines, min_val=0, max_val=7,
                                      skip_runtime_bounds_check=True)

            g_n1 = {u: ld(u, [SP]) for u in range(1, 5)}
            g_n2 = {u: ld(8 + u, [SP, PE]) for u in range(4, 8)}
            slot_n1 = {u: ld(u, [SP]) for u in range(1, 5)}
            slot_n2 = {u: ld(8 + u, [SP]) for u in range(4, 8)}
            my_slot = ld(16, [SP])

            rs_in_r = rs_in[:]  # [8, 3, NB]

            # es + sim-psum pools open for the whole kernel so the
            # diagonal sim units can overlap view-2 projection (PSUM:
            # proj 3+1 banks + sim 4 banks = 8).
            es_pool_cm = tc.tile_pool(name="es", bufs=12)
            sp_pool_cm = tc.tile_pool(name="sim_psum", bufs=2, space="PSUM")
            esp = es_pool_cm.__enter__()
            sp = sp_pool_cm.__enter__()

            # ================= projection phase =================
            with tc.tile_pool(name="proj", bufs=1) as proj, \
                 tc.tile_pool(name="ptmp", bufs=3) as ptmp, \
                 tc.tile_pool(name="ptv", bufs=1) as ptv, \
                 tc.tile_pool(name="ppsum", bufs=3, space="PSUM") as ppsum, \
                 tc.tile_pool(name="spsum", bufs=1, space="PSUM") as spsum:

                # zero the RS staging early (some slots are never written)
                zrow = ptv.tile([1, 3 * NB], F32)
                nc.vector.memset(zrow[:], 0.0)
                for s in range(N_CORES):
                    nc.sync.dma_start(
                        rs_in[s].rearrange("c m -> (c m)").unsqueeze(0),
                        zrow[:])

                zt_sb = [proj.tile([128, KT, NB], F16, name=f"zt{v}_sb")
                         for v in range(2)]
                w1_sb = proj.tile([128, KT, D], F16)
                w2_sb = proj.tile([128, KT, D], F16)
                b1_sb = proj.tile([128, KT], F32)
                b2_sb = proj.tile([128, KT], F32)
                hsq = proj.tile([128, KT, NB], F16)
                hh32 = proj.tile([128, KT, NB], F32)
                e_sb = proj.tile([128, KT, NB], F16)
                h_sb = [proj.tile([128, KT, NB], F32, name=f"h{v}")
                        for v in range(2)]

                nc.sync.dma_start(w1_sb[:], w1t)
                nc.sync.dma_start(zt_sb[0][:], zt1)
                nc.sync.dma_start(b1_sb[:], b1)
                nc.sync.dma_start(w2_sb[:], w2t)
                nc.sync.dma_start(b2_sb[:], b2)
                nc.sync.dma_start(zt_sb[1][:], zt2)

                for v in range(2):
                    # ---- layer 1 + ELU ----
                    for pt in range(KT):
                        for ch in range(NB // NCHUNK):
                            ps = ppsum.tile([128, NCHUNK], F32, tag="ps_proj")
                            for kt in range(KT):
                                nc.tensor.matmul(
                                    ps[:],
                                    lhsT=w1_sb[:, kt, pt * 128:(pt + 1) * 128],
                                    rhs=zt_sb[v][:, kt,
                                                 ch * NCHUNK:(ch + 1) * NCHUNK],
                                    start=(kt == 0), stop=(kt == KT - 1))
                            # elu(y) = relu(y) + min(exp(y),1) - 1, y = ps+b1
                            texp = ptmp.tile([128, NCHUNK], F16, tag="texp")
                            nc.scalar.activation(texp[:], ps[:], AF.Exp,
                                                 bias=b1_sb[:, pt:pt + 1],
                                                 scale=1.0)
                            tmax = ptmp.tile([128, NCHUNK], F16, tag="tmax")
                            nc.scalar.activation(tmax[:], ps[:], AF.Relu,
                                                 bias=b1_sb[:, pt:pt + 1],
                                                 scale=1.0)
                            tclip = ptmp.tile([128, NCHUNK], F16, tag="tclip")
                            nc.vector.tensor_scalar(tclip[:], texp[:], 1.0, -1.0,
                                                    ALU.min, ALU.add)
                            nc.vector.tensor_tensor(
                                e_sb[:, pt, ch * NCHUNK:(ch + 1) * NCHUNK],
                                tmax[:], tclip[:], ALU.add)
                    # ---- layer 2 (+ b2) ----
                    for jt in range(KT):
                        for ch in range(NB // NCHUNK):
                            ps = ppsum.tile([128, NCHUNK], F32, tag="ps_proj")
                            for kt in range(KT):
                                nc.tensor.matmul(
                                    ps[:],
                                    lhsT=w2_sb[:, kt, jt * 128:(jt + 1) * 128],
                                    rhs=e_sb[:, kt, ch * NCHUNK:(ch + 1) * NCHUNK],
                                    start=(kt == 0), stop=(kt == KT - 1))
                            sl = (slice(None), jt,
                                  slice(ch * NCHUNK, (ch + 1) * NCHUNK))
                            nc.scalar.activation(h_sb[v][sl], ps[:], AF.Identity,
                                                 bias=b2_sb[:, jt:jt + 1],
                                                 scale=1.0)
                            nc.vector.tensor_tensor(hsq[sl], h_sb[v][sl],
                                                    h_sb[v][sl], ALU.mult)
                    # ---- ss[i] = sum_f h[f,i]^2 via ones-matmul ----
                    ss = ptv.tile([1, NB], F32, tag="ss")
                    for ch in range(NB // NCHUNK):
                        pss = spsum.tile([1, NCHUNK], F32, tag="ps_small")
                        for jt in range(KT):
                            nc.tensor.matmul(pss[:], lhsT=ones_h[:],
                                             rhs=hsq[:, jt,
                                                     ch * NCHUNK:(ch + 1) * NCHUNK],
                                             start=(jt == 0), stop=(jt == KT - 1))
                        nc.vector.tensor_copy(ss[:, ch * NCHUNK:(ch + 1) * NCHUNK],
                                              pss[:])
                    # rn = rsqrt(ss): bit-trick seed + 1 Newton step (all DVE,
                    # no act-table switch).  y0 = bits(magic - (ss>>1));
                    # rn = y0*(1.5 - 0.5*ss*y0^2)
                    yb = ptv.tile([1, NB], U32, tag="yb")
                    nc.vector.tensor_scalar(yb[:], ss[:].bitcast(U32), -0.5,
                                            float(RSQRT_MAGIC) + 0.5,
                                            ALU.mult, ALU.add)
                    y0 = yb[:].bitcast(F32)
                    t1 = ptv.tile([1, NB], F32, tag="t1")
                    nc.vector.tensor_tensor(t1[:], y0, y0, ALU.mult)
                    nc.vector.tensor_tensor(t1[:], t1[:], ss[:], ALU.mult)
                    nc.vector.tensor_scalar(t1[:], t1[:], -0.5, 1.5,
                                            ALU.mult, ALU.add)
                    nc.vector.tensor_tensor(rn_vec[v][:], y0, t1[:],
                                            ALU.mult)

                    # broadcast rn across partitions (K=1 ones-matmul), scale
                    for ch in range(NB // NCHUNK):
                        pbc = spsum.tile([128, NCHUNK], F32, tag="ps_small")
                        nc.tensor.matmul(
                            pbc[:], lhsT=ones_row[:],
                            rhs=rn_vec[v][:, ch * NCHUNK:(ch + 1) * NCHUNK],
                            start=True, stop=True)
                        for jt in range(KT):
                            sl = (slice(None), jt,
                                  slice(ch * NCHUNK, (ch + 1) * NCHUNK))
                            nc.vector.tensor_tensor(n_loc[v][sl], h_sb[v][sl],
                                                    pbc[:], ALU.mult)

                    # ship to DRAM + AllGather (TOPSP+SDMA, overlaps compute)
                    nc.sync.dma_start(
                        cc_in[v][:],
                        n_loc[v][:].rearrange("p a b -> p (a b)"))
                    nc.gpsimd.collective_compute(
                        "AllGather", ALU.bypass, replica_groups=rg,
                        ins=[cc_in[v].opt()], outs=[cc_out[v].opt()])

                # ---- pos diagonal: s12_ii = rn1_i*rn2_i*sum_f h1[f,i]h2[f,i]
                hh = hh32
                for jt in range(KT):
                    nc.vector.tensor_tensor(hh[:, jt, :], h_sb[0][:, jt, :],
                                            h_sb[1][:, jt, :], ALU.mult)
                pos_part = ptv.tile([1, NB], F32, tag="pos_part")
                for ch in range(NB // NCHUNK):
                    psp = spsum.tile([1, NCHUNK], F32, tag="ps_small")
                    for jt in range(KT):
                        nc.tensor.matmul(psp[:], lhsT=ones_col[:],
                                         rhs=hh[:, jt,
                                                ch * NCHUNK:(ch + 1) * NCHUNK],
                                         start=(jt == 0), stop=(jt == KT - 1))
                    sl = (slice(0, 1), slice(ch * NCHUNK, (ch + 1) * NCHUNK))
                    nc.vector.tensor_tensor(pos_part[sl], psp[:], rn_vec[0][sl],
                                            ALU.mult)
                    nc.vector.tensor_tensor(pos_part[sl], pos_part[sl],
                                            rn_vec[1][sl], ALU.mult)
                nc.vector.tensor_reduce(pos_sum[:], pos_part[:],
                                        mybir.AxisListType.X, ALU.add)

                # ---- diagonal sim units (local data only; hide the AGs) ----
                # emitted inside the proj pool scope so they use proj PSUM
                # budget?  No - they need their own pool; open sim psum pool
                # here is not possible (pool nesting).  Diag units are emitted
                # in the sim section below but depend only on n_loc, so the
                # Tile scheduler can run them as soon as n_loc is ready.

            # ================= similarity phase =================
            # unit list: (key, lhs view, rhs kind, rhs index, bias, parts,
            #             parts slot, colsum dest)
            # rhs kind: 'loc' (n_loc[view]), 'n1' (dyn block of cc_out[0]),
            #           'n2s' (static block j of n2_sb), 'n2d' (dyn block)
            with tc.tile_pool(name="sim", bufs=1) as sim, \
                 tc.tile_pool(name="cs", bufs=6) as csp, \
                 tc.tile_pool(name="csum_psum", bufs=2, space="PSUM") as cp:

                # gathered n2, fully SBUF-resident (shared by S12 + S22
                # units); [p, kt, block, m] fp8
                n2_sb = sim.tile([128, KT, N_CORES, NB], F8, name="n2_sb")
                # dyn-loaded n1 blocks for S11 off-diag units u=1..4 and
                # n2 blocks for S22 units u=4..7 (static matmul operands;
                # only the DMA source offsets are dynamic)
                rhs1 = {u: sim.tile([128, KT, NB], F8, name=f"rhs1_{u}")
                        for u in range(1, 5)}
                rhs2 = {u: sim.tile([128, KT, NB], F8, name=f"rhs2_{u}")
                        for u in range(4, 8)}

                cc0r = cc_out[0][:].rearrange(
                    "(g p) (a m) -> g p a m", p=128, a=KT)
                cc1r = cc_out[1][:].rearrange(
                    "(g p) (a m) -> g p a m", p=128, a=KT)
                for u in range(1, 5):
                    nc.sync.dma_start(rhs1[u][:], cc0r[g_n1[u]])
                for j in range(N_CORES):
                    nc.sync.dma_start(n2_sb[:, :, j, :], cc1r[j])
                for u in range(4, 8):
                    nc.sync.dma_start(rhs2[u][:], cc1r[g_n2[u]])

                # pending colsum pass emitters, delayed by one unit
                pending = []

                def emit_pending():
                    while pending:
                        pending.pop(0)()

                def sim_unit(lhs, rhs_fn, bias, parts, pslot, colsum_dst,
                             keep_es, act_accum=False):
                    """One [NB x NB] unit: 8 row tiles x (4 DR MMs + exp ACT).
                    rhs_fn(kt2, c0, c1) -> AP of [128, c1-c0, 2] moving pairs.
                    colsum_dst: None or a dram AP receiving [1, NB]."""
                    es_tiles = []
                    for mt in range(MT):
                        ps = sp.tile([128, NB], F32, tag="ps_sim")
                        for kt2 in range(KT2):
                            for ck in range(2):
                                nc.tensor.matmul(
                                    ps[:, ck * NCHUNK:(ck + 1) * NCHUNK],
                                    lhsT=lhs[:, 2 * kt2:2 * kt2 + 2,
                                             mt * 128:(mt + 1) * 128],
                                    rhs=rhs_fn(kt2, ck * NCHUNK,
                                               (ck + 1) * NCHUNK),
                                    start=(kt2 == 0), stop=(kt2 == KT2 - 1),
                                    perf_mode=DR)
                        es = esp.tile([128, NB], F16, tag="es")
                        if keep_es:
                            es_tiles.append(es)
                        if act_accum:
                            nc.scalar.activation(
                                es[:], ps[:], AF.Exp,
                                scale=TAU_INV / (F8SCALE * F8SCALE), bias=bias,
                                accum_out=parts[:, mt, pslot:pslot + 1])
                        else:
                            nc.scalar.activation(
                                es[:], ps[:], AF.Exp,
                                scale=TAU_INV / (F8SCALE * F8SCALE), bias=bias)
                            nc.vector.tensor_reduce(
                                parts[:, mt, pslot:pslot + 1], es[:],
                                mybir.AxisListType.X, ALU.add)
                        # emit the previous unit's colsum pass midway
                        if mt == 3:
                            emit_pending()

                    if colsum_dst is not None:
                        def colsum_pass(es_tiles=es_tiles,
                                        colsum_dst=colsum_dst):
                            cs = csp.tile([1, NB], F32, tag="cs")
                            for ck in range(2):
                                psc = cp.tile([1, NCHUNK], F32, tag="psc")
                                for mt in range(MT):
                                    nc.tensor.matmul(
                                        psc[:],
                                        lhsT=ones_h[:],
                                        rhs=es_tiles[mt][:,
                                                         ck * NCHUNK:
                                                         (ck + 1) * NCHUNK],
                                        start=(mt == 0), stop=(mt == MT - 1),
                                        skip_group_check=True)
                                nc.vector.tensor_copy(
                                    cs[:, ck * NCHUNK:(ck + 1) * NCHUNK],
                                    psc[:])
                            nc.sync.dma_start(colsum_dst, cs[:])
                        pending.append(colsum_pass)

                def loc_rhs(v):
                    return lambda kt2, c0, c1: \
                        n_loc[v][:, 2 * kt2:2 * kt2 + 2, c0:c1]

                def rhs1_rhs(u):
                    return lambda kt2, c0, c1: \
                        rhs1[u][:, 2 * kt2:2 * kt2 + 2, c0:c1]

                def n2s_rhs(j):
                    return lambda kt2, c0, c1: \
                        n2_sb[:, 2 * kt2:2 * kt2 + 2, j, c0:c1]

                def n2d_rhs(u):
                    return lambda kt2, c0, c1: \
                        rhs2[u][:, 2 * kt2:2 * kt2 + 2, c0:c1]

                # --- diagonal units (only need n_loc; run under the AGs) ---
                sim_unit(n_loc[0], loc_rhs(0), 0.0, parts11, 0, None,
                         False, act_accum=True)
                sim_unit(n_loc[1], loc_rhs(1), 0.0, parts22, 0, None,
                         False, act_accum=True)

                # --- S11 off-diagonal units u=1..4 (need AG1) ---
                for u in range(1, 5):
                    bias = negln2 if u == 4 else 0.0
                    dst = rs_in_r[slot_n1[u], 0, :].unsqueeze(0)
                    sim_unit(n_loc[0], rhs1_rhs(u), bias, parts11, u, dst,
                             True)

                # --- S12 units j=0..7 (need AG2) ---
                for j in range(N_CORES):
                    dst = rs_in_r[j, 2, :].unsqueeze(0)
                    sim_unit(n_loc[0], n2s_rhs(j), 0.0, parts12, j, dst, True)

                # --- S22 off-diagonal units u=4..7 (need AG2) ---
                for u in range(4, 8):
                    bias = negln2 if u == 4 else 0.0
                    dst = rs_in_r[slot_n2[u], 1, :].unsqueeze(0)
                    sim_unit(n_loc[1], n2d_rhs(u), bias, parts22,
                             u - 3, dst, True, act_accum=True)

                emit_pending()

                # --- my-rows rowsum partials stay LOCAL (only colsums
                # need the cross-core reduce) ---
                my11 = sim.tile([128, MT], F32)
                my22 = sim.tile([128, MT], F32)
                rs12l = sim.tile([128, MT], F32)
                nc.vector.tensor_reduce(my11[:], parts11[:],
                                        mybir.AxisListType.X, ALU.add)
                nc.vector.tensor_reduce(my22[:], parts22[:],
                                        mybir.AxisListType.X, ALU.add)
                nc.vector.tensor_reduce(rs12l[:], parts12[:],
                                        mybir.AxisListType.X, ALU.add)

                if DEBUG:
                    for v in range(2):
                        nc.sync.dma_start(dbg_rn[v:v + 1, :], rn_vec[v][:])

                # dummy Ln: pulls the ln act-table load into the RS window
                dln = sim.tile([1, 1], F32)
                nc.scalar.activation(dln[:], ones_col[0:1, 0:1], AF.Ln)

                # --- ReduceScatter: slot c -> colsum contributions for my
                # rows (class 0: S11, 1: S22, 2: S21) ---
                nc.gpsimd.collective_compute(
                    "ReduceScatter", ALU.add, replica_groups=rg,
                    ins=[rs_in.opt()], outs=[rs_out.opt()])

                # --- final assembly ---
                # contiguous load [24, 128] then transpose via identity matmul
                rsC = sim.tile([24, 128], F32)
                nc.sync.dma_start(
                    rsC[:], rs_out[:].rearrange("c (mt p) -> (c mt) p", p=128))
                d12 = sim.tile([128, 2 * MT], F32)
                d1 = d12[:, 0:MT]
                d2 = d12[:, MT:2 * MT]
                with tc.tile_pool(name="fin_psum", bufs=1,
                                  space="PSUM") as fp:
                    ptr = fp.tile([128, 24], F32)
                    nc.tensor.matmul(ptr[:], lhsT=rsC[:],
                                     rhs=id_sb[0:24, 0:24],
                                     start=True, stop=True)
                    # totals (for debug + denominators)
                    t0 = sim.tile([128, MT], F32)
                    t1d = sim.tile([128, MT], F32)
                    nc.vector.tensor_tensor(t0[:], ptr[:, 0:MT], my11[:],
                                            ALU.add)
                    nc.vector.tensor_tensor(t1d[:], ptr[:, MT:2 * MT],
                                            my22[:], ALU.add)
                    nc.vector.tensor_tensor(d1, t0[:], rs12l[:], ALU.add)
                    nc.vector.tensor_tensor(d2, t1d[:],
                                            ptr[:, 2 * MT:3 * MT], ALU.add)
                    nc.vector.tensor_scalar_add(d12[:], d12[:], -E2)
                    if DEBUG:
                        t2 = sim.tile([128, MT], F32)
                        nc.vector.tensor_copy(t2[:], ptr[:, 2 * MT:3 * MT])
                        nc.sync.dma_start(dbg_rs[:, 0:MT], t0[:])
                        nc.sync.dma_start(dbg_rs[:, MT:2 * MT], t1d[:])
                        nc.sync.dma_start(dbg_rs[:, 2 * MT:3 * MT], t2[:])
                        nc.sync.dma_start(dbg_d[:, 0:2 * MT], d12[:])
                        nc.sync.dma_start(dbg_d[:, 2 * MT:3 * MT], rs12l[:])
                        nc.sync.dma_start(dbg_d[:, 3 * MT:4 * MT], my11[:])
                    nc.scalar.activation(d12[:], d12[:], AF.Ln)
                    lsum = sim.tile([128, MT], F32)
                    nc.vector.tensor_tensor(lsum[:], d1, d2, ALU.add)
                    lrow = sim.tile([128, 1], F32)
                    nc.vector.tensor_reduce(lrow[:], lsum[:],
                                            mybir.AxisListType.X, ALU.add)
                    pfin = fp.tile([1, 1], F32)
                    nc.tensor.matmul(pfin[:], lhsT=ones_col[:], rhs=lrow[:],
                                     start=True, stop=True)
                    fin = sim.tile([1, 1], F32)
                    nc.vector.tensor_scalar_mul(fin[:], pfin[:], 0.5)
                    p2 = sim.tile([1, 1], F32)
                    nc.vector.tensor_scalar_mul(p2[:], pos_sum[:], 2.0)
                    nc.vector.tensor_tensor(fin[:], fin[:], p2[:],
                                            ALU.subtract)
                    nc.sync.dma_start(out, fin[:])

            sp_pool_cm.__exit__(None, None, None)
            es_pool_cm.__exit__(None, None, None)

    nc.compile()
    return nc


def _prep_inputs(z1, z2, fc1_w, fc1_b, fc2_w, fc2_b):
    """Host-side shard + layout prep. Returns in_maps for the 8 cores."""
    w1t = np.ascontiguousarray(fc1_w.T).reshape(KT, 128, D).transpose(1, 0, 2)
    w1t = np.ascontiguousarray(w1t, dtype=np.float16)
    w2t = np.ascontiguousarray(fc2_w.T).reshape(KT, 128, D).transpose(1, 0, 2)
    w2t = np.ascontiguousarray(w2t, dtype=np.float16)
    b1 = np.ascontiguousarray(fc1_b.reshape(KT, 128).T, dtype=np.float32)
    b2 = np.ascontiguousarray(fc2_b.reshape(KT, 128).T, dtype=np.float32)

    in_maps = []
    for c in range(N_CORES):
        blk1 = z1[c * NB:(c + 1) * NB].T            # [512, 1024]
        blk2 = z2[c * NB:(c + 1) * NB].T
        zt1 = np.ascontiguousarray(
            blk1.reshape(KT, 128, NB).transpose(1, 0, 2), dtype=np.float16)
        zt2 = np.ascontiguousarray(
            blk2.reshape(KT, 128, NB).transpose(1, 0, 2), dtype=np.float16)
        gidx = np.zeros((3, 8), np.uint32)
        for u in range(1, 5):
            gidx[0, u] = (c + u) % N_CORES
        for u in range(4, 8):
            gidx[1, u] = (c + u) % N_CORES
        gidx[2, 0] = c
        in_maps.append({"zt1": zt1, "zt2": zt2, "w1t": w1t, "w2t": w2t,
                        "b1": b1, "b2": b2, "gidx": gidx,
                        "ident": np.eye(128, dtype=np.float32)})
    return in_maps


def kernel(z1, z2, fc1_w, fc1_b, fc2_w, fc2_b):
    global LAST_EXEC_NS
    z1 = np.asarray(z1, dtype=np.float32)
    z2 = np.asarray(z2, dtype=np.float32)
    fc1_w = np.asarray(fc1_w, dtype=np.float32)
    fc1_b = np.asarray(fc1_b, dtype=np.float32)
    fc2_w = np.asarray(fc2_w, dtype=np.float32)
    fc2_b = np.asarray(fc2_b, dtype=np.float32)

    if "nc" not in _CACHE:
        _CACHE["nc"] = _build_program()
    nc = _CACHE["nc"]

    in_maps = _prep_inputs(z1, z2, fc1_w, fc1_b, fc2_w, fc2_b)
    res = run_bass_kernel_spmd(nc, in_maps, core_ids=list(range(N_CORES)),
                               trace=TRACE)
    LAST_EXEC_NS = res.exec_time_ns
    total = math.fsum(float(r["out"][0, 0]) for r in res.results)
    return np.float32(total / N)


# revision 27
# speedup vs baseline: 1.0230x; 1.0230x over previous
"""GRACE contrastive loss kernel for Trainium2 (8 NeuronCores, SPMD).

Strategy (row-block data parallel + symmetry harvesting):
  - Shard the N=8192 nodes across 8 cores (NB=1024 rows each).  Each core
    projects its z1/z2 block through the 2-layer MLP (fp16 matmuls, fp32
    accum), row-normalizes (DVE bit-trick rsqrt, no act-table switch), and
    AllGathers the normalized embeddings (fp16).
  - Similarity work per core is 18 block-units of [1024 x 1024] x K=512
    (vs 32 for the naive 4-matrix scheme):
      * S12 (between_sim): all 8 column blocks — row sums via exp accum_out;
        column sums (ones-matmuls over the exp tiles) give the row sums of
        S21 = S12^T, so S21 is never materialized.
      * S11/S22 (refl_sims): diagonal block locally (no gather needed, runs
        under the AllGathers), plus a shift-invariant triangular assignment:
        unit u in {1,2,3} -> S11 block (c+u)%8, u in {5,6,7} -> S22 block
        (c+u)%8, and u=4 computed by BOTH pair members for BOTH matrices
        with exp pre-halved via bias=-ln(2).  Column sums of each
        off-diagonal exp tile are the transposed block's row-sum
        contribution.
  - All cross-core terms meet in one ReduceScatter over [slot(8) x class(3)
    x 1024] f32; slot c returns exactly core c's total S11/S22/S21 row sums.
    Core-dependent gather/scatter positions use dynamic (register) offsets
    read from tiny per-core uint32 index tables.
"""

import math
import sys

import numpy as np

sys.path.insert(0, "/opt/trn_rl_repo")

import concourse.bass as bass  # noqa: E402
import concourse.mybir as mybir  # noqa: E402
import concourse.tile as tile  # noqa: E402
from concourse import bacc  # noqa: E402
from concourse.bass_utils import run_bass_kernel_spmd  # noqa: E402

F32 = mybir.dt.float32
F16 = mybir.dt.float16
F8 = mybir.dt.float8e4
U32 = mybir.dt.uint32
DR = mybir.MatmulPerfMode.DoubleRow
AF = mybir.ActivationFunctionType
ALU = mybir.AluOpType
SP = mybir.EngineType.SP
PE = mybir.EngineType.PE

N_CORES = 8
N = 8192
D = 512            # feature dim (= H = P in the reference MLP)
NB = N // N_CORES  # 1024 rows per core
KT = D // 128      # 4 k-subtiles
KT2 = KT // 2      # 2 DoubleRow k-subtiles (K=256 each)
F8SCALE = 16.0     # embeddings are shipped as fp8e4 * F8SCALE
MT = NB // 128     # 8 row tiles per core
NCHUNK = 512       # matmul moving free dim (one PSUM bank)
TAU_INV = 2.0      # 1 / tau
E2 = float(np.exp(2.0, dtype=np.float64))  # exp(diag(refl_sim)/tau)
LN2 = float(np.log(2.0))
RSQRT_MAGIC = 0x5F3759DF

TRACE = False
DEBUG = False
LAST_EXEC_NS = None
_CACHE = {}


def _build_program():
    nc = bacc.Bacc("TRN2", target_bir_lowering=False, debug=False,
                   num_devices=N_CORES)

    # ---- I/O ----
    zt1 = nc.dram_tensor("zt1", [128, KT, NB], F16, kind="ExternalInput").ap()
    zt2 = nc.dram_tensor("zt2", [128, KT, NB], F16, kind="ExternalInput").ap()
    w1t = nc.dram_tensor("w1t", [128, KT, D], F16, kind="ExternalInput").ap()
    w2t = nc.dram_tensor("w2t", [128, KT, D], F16, kind="ExternalInput").ap()
    b1 = nc.dram_tensor("b1", [128, KT], F32, kind="ExternalInput").ap()
    b2 = nc.dram_tensor("b2", [128, KT], F32, kind="ExternalInput").ap()
    # per-core index tables (uint32):
    #   gidx[0, u] u=1..4 : gather block (c+u)%8 in the n1 half
    #   gidx[1, u] u=4..7 : gather block (c+u)%8 in the n2 half
    #   gidx[2, 0]        : my slot c
    gidx = nc.dram_tensor("gidx", [3, 8], U32, kind="ExternalInput").ap()
    ident = nc.dram_tensor("ident", [128, 128], F32, kind="ExternalInput").ap()
    out = nc.dram_tensor("out", [1, 1], F32, kind="ExternalOutput").ap()
    if DEBUG:
        dbg_rn = nc.dram_tensor("dbg_rn", [2, NB], F32,
                                kind="ExternalOutput").ap()
        dbg_rs = nc.dram_tensor("dbg_rs", [128, 3 * MT], F32,
                                kind="ExternalOutput").ap()
        dbg_d = nc.dram_tensor("dbg_d", [128, 4 * MT], F32,
                               kind="ExternalOutput").ap()

    rg = [list(range(N_CORES))]

    with tile.TileContext(nc) as tc:
        with tc.tile_pool(name="persist", bufs=1) as persist, \
             tc.tile_pool(name="dram", bufs=1, space="DRAM") as dram:

            ones_col = persist.tile([128, 1], F32)
            nc.vector.memset(ones_col[:], 1.0)
            ones_h = persist.tile([128, 1], F16)
            nc.vector.memset(ones_h[:], 1.0)
            ones_row = persist.tile([1, 128], F32)
            nc.vector.memset(ones_row[:], F8SCALE)
            negln2 = persist.tile([128, 1], F32)
            nc.vector.memset(negln2[:], -LN2)
            id_sb = persist.tile([128, 128], F32, name="id_sb")
            nc.sync.dma_start(id_sb[:], ident)

            # local normalized blocks [feature, node] fp8 * F8SCALE;
            # DoubleRow reads kt-pairs as strided [Ki, Ko=2, *] planes
            n_loc = [persist.tile([128, KT, NB], F8, name=f"n{v}_loc")
                     for v in range(2)]
            rn_vec = [persist.tile([1, NB], F32, name=f"rn{v}") for v in range(2)]

            # ship layout = the SBUF n_loc layout, flattened: [128, 4096] f8
            cc_in = [dram.tile([128, KT * NB], F8, name=f"cc_in{v}")
                     for v in range(2)]
            cc_out = [dram.tile([N_CORES * 128, KT * NB], F8,
                                name=f"cc_out{v}", addr_space="Shared",
                                tag=("agbuf0" if v == 0 else "agbuf1"))
                      for v in range(2)]

            # ReduceScatter staging [slot, class, row] f32;
            # class 0: S11 rowsums, 1: S22 rowsums, 2: S21 rowsums (=colsum S12)
            rs_in = dram.tile([N_CORES, 3, NB], F32, name="rs_in")
            rs_out = dram.tile([3, NB], F32, name="rs_out")

            # rowsum partials from exp accum_out
            parts12 = persist.tile([128, MT, 8], F32, name="parts12")
            parts11 = persist.tile([128, MT, 5], F32, name="parts11")
            parts22 = persist.tile([128, MT, 5], F32, name="parts22")
            pos_sum = persist.tile([1, 1], F32)

            # ---- dynamic index registers ----
            # engines can't read DRAM and the runtime bounds-check assert is
            # broken under this runtime: stage the table in SBUF and skip
            # the runtime check (bounds are guaranteed host-side).
            gidx_sb = persist.tile([1, 24], U32, name="gidx_sb")
            nc.sync.dma_start(gidx_sb[:],
                              gidx[:].rearrange("a b -> (a b)").unsqueeze(0))

            def ld(flat, engines):
                return nc.values_load(gidx_sb[0:1, flat:flat + 1],
                                      engines=engines, min_val=0, max_val=7,
                                      skip_runtime_bounds_check=True)

            g_n1 = {u: ld(u, [SP]) for u in range(1, 5)}
            g_n2 = {u: ld(8 + u, [SP, PE]) for u in range(4, 8)}
            slot_n1 = {u: ld(u, [SP]) for u in range(1, 5)}
            slot_n2 = {u: ld(8 + u, [SP]) for u in range(4, 8)}
            my_slot = ld(16, [SP])

            rs_in_r = rs_in[:]  # [8, 3, NB]

            # es + sim-psum pools open for the whole kernel so the
            # diagonal sim units can overlap view-2 projection (PSUM:
            # proj 3+1 banks + sim 4 banks = 8).
            es_pool_cm = tc.tile_pool(name="es", bufs=12)
            sp_pool_cm = tc.tile_pool(name="sim_psum", bufs=2, space="PSUM")
            esp = es_pool_cm.__enter__()
            sp = sp_pool_cm.__enter__()

            # ================= projection phase =================
            with tc.tile_pool(name="proj", bufs=1) as proj, \
                 tc.tile_pool(name="ptmp", bufs=3) as ptmp, \
                 tc.tile_pool(name="ptv", bufs=1) as ptv, \
                 tc.tile_pool(name="ppsum", bufs=3, space="PSUM") as ppsum, \
                 tc.tile_pool(name="spsum", bufs=1, space="PSUM") as spsum:

                # zero the RS staging early (some slots are never written)
                zrow = ptv.tile([1, 3 * NB], F32)
                nc.vector.memset(zrow[:], 0.0)
                for s in range(N_CORES):
                    nc.sync.dma_start(
                        rs_in[s].rearrange("c m -> (c m)").unsqueeze(0),
                        zrow[:])

                zt_sb = [proj.tile([128, KT, NB], F16, name=f"zt{v}_sb")
                         for v in range(2)]
                w1_sb = proj.tile([128, KT, D], F16)
                w2_sb = proj.tile([128, KT, D], F16)
                b1_sb = proj.tile([128, KT], F32)
                b2_sb = proj.tile([128, KT], F32)
                hsq = proj.tile([128, KT, NB], F16)
                hh32 = proj.tile([128, KT, NB], F32)
                e_sb = proj.tile([128, KT, NB], F16)
                h_sb = [proj.tile([128, KT, NB], F32, name=f"h{v}")
                        for v in range(2)]

                nc.sync.dma_start(w1_sb[:], w1t)
                nc.sync.dma_start(zt_sb[0][:], zt1)
                nc.sync.dma_start(b1_sb[:], b1)
                nc.sync.dma_start(w2_sb[:], w2t)
                nc.sync.dma_start(b2_sb[:], b2)
                nc.sync.dma_start(zt_sb[1][:], zt2)

                for v in range(2):
                    # ---- layer 1 + ELU ----
                    for pt in range(KT):
                        for ch in range(NB // NCHUNK):
                            ps = ppsum.tile([128, NCHUNK], F32, tag="ps_proj")
                            for kt in range(KT):
                                nc.tensor.matmul(
                                    ps[:],
                                    lhsT=w1_sb[:, kt, pt * 128:(pt + 1) * 128],
                                    rhs=zt_sb[v][:, kt,
                                                 ch * NCHUNK:(ch + 1) * NCHUNK],
                                    start=(kt == 0), stop=(kt == KT - 1))
                            # elu(y) = relu(y) + min(exp(y),1) - 1, y = ps+b1
                            texp = ptmp.tile([128, NCHUNK], F16, tag="texp")
                            nc.scalar.activation(texp[:], ps[:], AF.Exp,
                                                 bias=b1_sb[:, pt:pt + 1],
                                                 scale=1.0)
                            tmax = ptmp.tile([128, NCHUNK], F16, tag="tmax")
                            nc.scalar.activation(tmax[:], ps[:], AF.Relu,
                                                 bias=b1_sb[:, pt:pt + 1],
                                                 scale=1.0)
                            tclip = ptmp.tile([128, NCHUNK], F16, tag="tclip")
                            nc.vector.tensor_scalar(tclip[:], texp[:], 1.0, -1.0,
                                                    ALU.min, ALU.add)
                            nc.vector.tensor_tensor(
                                e_sb[:, pt, ch * NCHUNK:(ch + 1) * NCHUNK],
                                tmax[:], tclip[:], ALU.add)
                    # ---- layer 2 (+ b2) ----
                    for jt in range(KT):
                        for ch in range(NB // NCHUNK):
                            ps = ppsum.tile([128, NCHUNK], F32, tag="ps_proj")
                            for kt in range(KT):
                                nc.tensor.matmul(
                                    ps[:],
                                    lhsT=w2_sb[:, kt, jt * 128:(jt + 1) * 128],
                                    rhs=e_sb[:, kt, ch * NCHUNK:(ch + 1) * NCHUNK],
                                    start=(kt == 0), stop=(kt == KT - 1))
                            sl = (slice(None), jt,
                                  slice(ch * NCHUNK, (ch + 1) * NCHUNK))
                            nc.scalar.activation(h_sb[v][sl], ps[:], AF.Identity,
                                                 bias=b2_sb[:, jt:jt + 1],
                                                 scale=1.0)
                            nc.vector.tensor_tensor(hsq[sl], h_sb[v][sl],
                                                    h_sb[v][sl], ALU.mult)
                    # ---- ss[i] = sum_f h[f,i]^2 via ones-matmul ----
                    ss = ptv.tile([1, NB], F32, tag="ss")
                    for ch in range(NB // NCHUNK):
                        pss = spsum.tile([1, NCHUNK], F32, tag="ps_small")
                        for jt in range(KT):
                            nc.tensor.matmul(pss[:], lhsT=ones_h[:],
                                             rhs=hsq[:, jt,
                                                     ch * NCHUNK:(ch + 1) * NCHUNK],
                                             start=(jt == 0), stop=(jt == KT - 1))
                        nc.vector.tensor_copy(ss[:, ch * NCHUNK:(ch + 1) * NCHUNK],
                                              pss[:])
                    # rn = rsqrt(ss): bit-trick seed + 1 Newton step (all DVE,
                    # no act-table switch).  y0 = bits(magic - (ss>>1));
                    # rn = y0*(1.5 - 0.5*ss*y0^2)
                    yb = ptv.tile([1, NB], U32, tag="yb")
                    nc.vector.tensor_scalar(yb[:], ss[:].bitcast(U32), -0.5,
                                            float(RSQRT_MAGIC) + 0.5,
                                            ALU.mult, ALU.add)
                    y0 = yb[:].bitcast(F32)
                    t1 = ptv.tile([1, NB], F32, tag="t1")
                    nc.vector.tensor_tensor(t1[:], y0, y0, ALU.mult)
                    nc.vector.tensor_tensor(t1[:], t1[:], ss[:], ALU.mult)
                    nc.vector.tensor_scalar(t1[:], t1[:], -0.5, 1.5,
                                            ALU.mult, ALU.add)
                    nc.vector.tensor_tensor(rn_vec[v][:], y0, t1[:],
                                            ALU.mult)

                    # broadcast rn across partitions (K=1 ones-matmul), scale
                    for ch in range(NB // NCHUNK):
                        pbc = spsum.tile([128, NCHUNK], F32, tag="ps_small")
                        nc.tensor.matmul(
                            pbc[:], lhsT=ones_row[:],
                            rhs=rn_vec[v][:, ch * NCHUNK:(ch + 1) * NCHUNK],
                            start=True, stop=True)
                        for jt in range(KT):
                            sl = (slice(None), jt,
                                  slice(ch * NCHUNK, (ch + 1) * NCHUNK))
                            nc.vector.tensor_tensor(n_loc[v][sl], h_sb[v][sl],
                                                    pbc[:], ALU.mult)

                    # ship to DRAM + AllGather (TOPSP+SDMA, overlaps compute)
                    nc.sync.dma_start(
                        cc_in[v][:],
                        n_loc[v][:].rearrange("p a b -> p (a b)"))
                    nc.gpsimd.collective_compute(
                        "AllGather", ALU.bypass, replica_groups=rg,
                        ins=[cc_in[v].opt()], outs=[cc_out[v].opt()])

                # ---- pos diagonal: s12_ii = rn1_i*rn2_i*sum_f h1[f,i]h2[f,i]
                hh = hh32
                for jt in range(KT):
                    nc.vector.tensor_tensor(hh[:, jt, :], h_sb[0][:, jt, :],
                                            h_sb[1][:, jt, :], ALU.mult)
                pos_part = ptv.tile([1, NB], F32, tag="pos_part")
                for ch in range(NB // NCHUNK):
                    psp = spsum.tile([1, NCHUNK], F32, tag="ps_small")
                    for jt in range(KT):
                        nc.tensor.matmul(psp[:], lhsT=ones_col[:],
                                         rhs=hh[:, jt,
                                                ch * NCHUNK:(ch + 1) * NCHUNK],
                                         start=(jt == 0), stop=(jt == KT - 1))
                    sl = (slice(0, 1), slice(ch * NCHUNK, (ch + 1) * NCHUNK))
                    nc.vector.tensor_tensor(pos_part[sl], psp[:], rn_vec[0][sl],
                                            ALU.mult)
                    nc.vector.tensor_tensor(pos_part[sl], pos_part[sl],
                                            rn_vec[1][sl], ALU.mult)
                nc.vector.tensor_reduce(pos_sum[:], pos_part[:],
                                        mybir.AxisListType.X, ALU.add)

                # ---- diagonal sim units (local data only; hide the AGs) ----
                # emitted inside the proj pool scope so they use proj PSUM
                # budget?  No - they need their own pool; open sim psum pool
                # here is not possible (pool nesting).  Diag units are emitted
                # in the sim section below but depend only on n_loc, so the
                # Tile scheduler can run them as soon as n_loc is ready.

            # ================= similarity phase =================
            # unit list: (key, lhs view, rhs kind, rhs index, bias, parts,
            #             parts slot, colsum dest)
            # rhs kind: 'loc' (n_loc[view]), 'n1' (dyn block of cc_out[0]),
            #           'n2s' (static block j of n2_sb), 'n2d' (dyn block)
            with tc.tile_pool(name="sim", bufs=1) as sim, \
                 tc.tile_pool(name="cs", bufs=6) as csp, \
                 tc.tile_pool(name="csum_psum", bufs=2, space="PSUM") as cp:

                # gathered n2, fully SBUF-resident (shared by S12 + S22
                # units); [p, kt, block, m] fp8
                n2_sb = sim.tile([128, KT, N_CORES, NB], F8, name="n2_sb")
                # dyn-loaded n1 blocks for S11 off-diag units u=1..4 and
                # n2 blocks for S22 units u=4..7 (static matmul operands;
                # only the DMA source offsets are dynamic)
                rhs1 = {u: sim.tile([128, KT, NB], F8, name=f"rhs1_{u}")
                        for u in range(1, 5)}
                rhs2 = {u: sim.tile([128, KT, NB], F8, name=f"rhs2_{u}")
                        for u in range(4, 8)}

                cc0r = cc_out[0][:].rearrange(
                    "(g p) (a m) -> g p a m", p=128, a=KT)
                cc1r = cc_out[1][:].rearrange(
                    "(g p) (a m) -> g p a m", p=128, a=KT)
                for u in range(1, 5):
                    nc.sync.dma_start(rhs1[u][:], cc0r[g_n1[u]])
                for j in range(N_CORES):
                    nc.sync.dma_start(n2_sb[:, :, j, :], cc1r[j])
                for u in range(4, 8):
                    nc.sync.dma_start(rhs2[u][:], cc1r[g_n2[u]])

                # pending colsum pass emitters, delayed by one unit
                pending = []

                def emit_pending():
                    while pending:
                        pending.pop(0)()

                def sim_unit(lhs, rhs_fn, bias, parts, pslot, colsum_dst,
                             keep_es, act_accum=False):
                    """One [NB x NB] unit: 8 row tiles x (4 DR MMs + exp ACT).
                    rhs_fn(kt2, c0, c1) -> AP of [128, c1-c0, 2] moving pairs.
                    colsum_dst: None or a dram AP receiving [1, NB]."""
                    es_tiles = []
                    for mt in range(MT):
                        ps = sp.tile([128, NB], F32, tag="ps_sim")
                        for kt2 in range(KT2):
                            for ck in range(2):
                                nc.tensor.matmul(
                                    ps[:, ck * NCHUNK:(ck + 1) * NCHUNK],
                                    lhsT=lhs[:, 2 * kt2:2 * kt2 + 2,
                                             mt * 128:(mt + 1) * 128],
                                    rhs=rhs_fn(kt2, ck * NCHUNK,
                                               (ck + 1) * NCHUNK),
                                    start=(kt2 == 0), stop=(kt2 == KT2 - 1),
                                    perf_mode=DR)
                        es = esp.tile([128, NB], F16, tag="es")
                        if keep_es:
                            es_tiles.append(es)
                        if act_accum:
                            nc.scalar.activation(
                                es[:], ps[:], AF.Exp,
                                scale=TAU_INV / (F8SCALE * F8SCALE), bias=bias,
                                accum_out=parts[:, mt, pslot:pslot + 1])
                        else:
                            nc.scalar.activation(
                                es[:], ps[:], AF.Exp,
                                scale=TAU_INV / (F8SCALE * F8SCALE), bias=bias)
                            nc.vector.tensor_reduce(
                                parts[:, mt, pslot:pslot + 1], es[:],
                                mybir.AxisListType.X, ALU.add)
                        # emit the previous unit's colsum pass midway
                        if mt == 3:
                            emit_pending()

                    if colsum_dst is not None:
                        def colsum_pass(es_tiles=es_tiles,
                                        colsum_dst=colsum_dst):
                            cs = csp.tile([1, NB], F32, tag="cs")
                            for ck in range(2):
                                psc = cp.tile([1, NCHUNK], F32, tag="psc")
                                for mt in range(MT):
                                    nc.tensor.matmul(
                                        psc[:],
                                        lhsT=ones_h[:],
                                        rhs=es_tiles[mt][:,
                                                         ck * NCHUNK:
                                                         (ck + 1) * NCHUNK],
                                        start=(mt == 0), stop=(mt == MT - 1),
                                        skip_group_check=True)
                                nc.vector.tensor_copy(
                                    cs[:, ck * NCHUNK:(ck + 1) * NCHUNK],
                                    psc[:])
                            nc.sync.dma_start(colsum_dst, cs[:])
                        pending.append(colsum_pass)

                def loc_rhs(v):
                    return lambda kt2, c0, c1: \
                        n_loc[v][:, 2 * kt2:2 * kt2 + 2, c0:c1]

                def rhs1_rhs(u):
                    return lambda kt2, c0, c1: \
                        rhs1[u][:, 2 * kt2:2 * kt2 + 2, c0:c1]

                def n2s_rhs(j):
                    return lambda kt2, c0, c1: \
                        n2_sb[:, 2 * kt2:2 * kt2 + 2, j, c0:c1]

                def n2d_rhs(u):
                    return lambda kt2, c0, c1: \
                        rhs2[u][:, 2 * kt2:2 * kt2 + 2, c0:c1]

                # --- diagonal units (only need n_loc; run under the AGs) ---
                sim_unit(n_loc[0], loc_rhs(0), 0.0, parts11, 0, None,
                         False, act_accum=True)
                sim_unit(n_loc[1], loc_rhs(1), 0.0, parts22, 0, None,
                         False, act_accum=True)

                # --- S11 off-diagonal units u=1..4 (need AG1) ---
                for u in range(1, 5):
                    bias = negln2 if u == 4 else 0.0
                    dst = rs_in_r[slot_n1[u], 0, :].unsqueeze(0)
                    sim_unit(n_loc[0], rhs1_rhs(u), bias, parts11, u, dst,
                             True)

                # --- S12 units j=0..7 (need AG2) ---
                for j in range(N_CORES):
                    dst = rs_in_r[j, 2, :].unsqueeze(0)
                    sim_unit(n_loc[0], n2s_rhs(j), 0.0, parts12, j, dst, True)

                # --- S22 off-diagonal units u=4..7 (need AG2) ---
                for u in range(4, 8):
                    bias = negln2 if u == 4 else 0.0
                    dst = rs_in_r[slot_n2[u], 1, :].unsqueeze(0)
                    sim_unit(n_loc[1], n2d_rhs(u), bias, parts22,
                             u - 3, dst, True)

                emit_pending()

                # --- my-rows rowsum partials stay LOCAL (only colsums
                # need the cross-core reduce) ---
                my11 = sim.tile([128, MT], F32)
                my22 = sim.tile([128, MT], F32)
                rs12l = sim.tile([128, MT], F32)
                nc.vector.tensor_reduce(my11[:], parts11[:],
                                        mybir.AxisListType.X, ALU.add)
                nc.vector.tensor_reduce(my22[:], parts22[:],
                                        mybir.AxisListType.X, ALU.add)
                nc.vector.tensor_reduce(rs12l[:], parts12[:],
                                        mybir.AxisListType.X, ALU.add)

                if DEBUG:
                    for v in range(2):
                        nc.sync.dma_start(dbg_rn[v:v + 1, :], rn_vec[v][:])

                # dummy Ln: pulls the ln act-table load into the RS window
                dln = sim.tile([1, 1], F32)
                nc.scalar.activation(dln[:], ones_col[0:1, 0:1], AF.Ln)

                # --- ReduceScatter: slot c -> colsum contributions for my
                # rows (class 0: S11, 1: S22, 2: S21) ---
                nc.gpsimd.collective_compute(
                    "ReduceScatter", ALU.add, replica_groups=rg,
                    ins=[rs_in.opt()], outs=[rs_out.opt()])

                # --- final assembly ---
                # contiguous load [24, 128] then transpose via identity matmul
                rsC = sim.tile([24, 128], F32)
                nc.sync.dma_start(
                    rsC[:], rs_out[:].rearrange("c (mt p) -> (c mt) p", p=128))
                d12 = sim.tile([128, 2 * MT], F32)
                d1 = d12[:, 0:MT]
                d2 = d12[:, MT:2 * MT]
                with tc.tile_pool(name="fin_psum", bufs=1,
                                  space="PSUM") as fp:
                    ptr = fp.tile([128, 24], F32)
                    nc.tensor.matmul(ptr[:], lhsT=rsC[:],
                                     rhs=id_sb[0:24, 0:24],
                                     start=True, stop=True)
                    # totals (for debug + denominators)
                    t0 = sim.tile([128, MT], F32)
                    t1d = sim.tile([128, MT], F32)
                    nc.vector.tensor_tensor(t0[:], ptr[:, 0:MT], my11[:],
                                            ALU.add)
                    nc.vector.tensor_tensor(t1d[:], ptr[:, MT:2 * MT],
                                            my22[:], ALU.add)
                    nc.vector.tensor_tensor(d1, t0[:], rs12l[:], ALU.add)
                    nc.vector.tensor_tensor(d2, t1d[:],
                                            ptr[:, 2 * MT:3 * MT], ALU.add)
                    nc.vector.tensor_scalar_add(d12[:], d12[:], -E2)
                    if DEBUG:
                        t2 = sim.tile([128, MT], F32)
                        nc.vector.tensor_copy(t2[:], ptr[:, 2 * MT:3 * MT])
                        nc.sync.dma_start(dbg_rs[:, 0:MT], t0[:])
                        nc.sync.dma_start(dbg_rs[:, MT:2 * MT], t1d[:])
                        nc.sync.dma_start(dbg_rs[:, 2 * MT:3 * MT], t2[:])
                        nc.sync.dma_start(dbg_d[:, 0:2 * MT], d12[:])
                        nc.sync.dma_start(dbg_d[:, 2 * MT:3 * MT], rs12l[:])
                        nc.sync.dma_start(dbg_d[:, 3 * MT:4 * MT], my11[:])
                    nc.scalar.activation(d12[:], d12[:], AF.Ln)
                    lsum = sim.tile([128, MT], F32)
                    nc.vector.tensor_tensor(lsum[:], d1, d2, ALU.add)
                    lrow = sim.tile([128, 1], F32)
                    nc.vector.tensor_reduce(lrow[:], lsum[:],
                                            mybir.AxisListType.X, ALU.add)
                    pfin = fp.tile([1, 1], F32)
                    nc.tensor.matmul(pfin[:], lhsT=ones_col[:], rhs=lrow[:],
                                     start=True, stop=True)
                    fin = sim.tile([1, 1], F32)
                    nc.vector.tensor_scalar_mul(fin[:], pfin[:], 0.5)
                    p2 = sim.tile([1, 1], F32)
                    nc.vector.tensor_scalar_mul(p2[:], pos_sum[:], 2.0)
                    nc.vector.tensor_tensor(fin[:], fin[:], p2[:],
                                            ALU.subtract)
                    nc.sync.dma_start(out, fin[:])

            sp_pool_cm.__exit__(None, None, None)
            es_pool_cm.__exit__(None, None, None)

    nc.compile()
    return nc


def _prep_inputs(z1, z2, fc1_w, fc1_b, fc2_w, fc2_b):
    """Host-side shard + layout prep. Returns in_maps for the 8 cores."""
    w1t = np.ascontiguousarray(fc1_w.T).reshape(KT, 128, D).transpose(1, 0, 2)
    w1t = np.ascontiguousarray(w1t, dtype=np.float16)
    w2t = np.ascontiguousarray(fc2_w.T).reshape(KT, 128, D).transpose(1, 0, 2)
    w2t = np.ascontiguousarray(w2t, dtype=np.float16)
    b1 = np.ascontiguousarray(fc1_b.reshape(KT, 128).T, dtype=np.float32)
    b2 = np.ascontiguousarray(fc2_b.reshape(KT, 128).T, dtype=np.float32)

    in_maps = []
    for c in range(N_CORES):
        blk1 = z1[c * NB:(c + 1) * NB].T            # [512, 1024]
        blk2 = z2[c * NB:(c + 1) * NB].T
        zt1 = np.ascontiguousarray(
            blk1.reshape(KT, 128, NB).transpose(1, 0, 2), dtype=np.float16)
        zt2 = np.ascontiguousarray(
            blk2.reshape(KT, 128, NB).transpose(1, 0, 2), dtype=np.float16)
        gidx = np.zeros((3, 8), np.uint32)
        for u in range(1, 5):
            gidx[0, u] = (c + u) % N_CORES
        for u in range(4, 8):
            gidx[1, u] = (c + u) % N_CORES
        gidx[2, 0] = c
        in_maps.append({"zt1": zt1, "zt2": zt2, "w1t": w1t, "w2t": w2t,
                        "b1": b1, "b2": b2, "gidx": gidx,
                        "ident": np.eye(128, dtype=np.float32)})
    return in_maps


def kernel(z1, z2, fc1_w, fc1_b, fc2_w, fc2_b):
    global LAST_EXEC_NS
    z1 = np.asarray(z1, dtype=np.float32)
    z2 = np.asarray(z2, dtype=np.float32)
    fc1_w = np.asarray(fc1_w, dtype=np.float32)
    fc1_b = np.asarray(fc1_b, dtype=np.float32)
    fc2_w = np.asarray(fc2_w, dtype=np.float32)
    fc2_b = np.asarray(fc2_b, dtype=np.float32)

    if "nc" not in _CACHE:
        _CACHE["nc"] = _build_program()
    nc = _CACHE["nc"]

    in_maps = _prep_inputs(z1, z2, fc1_w, fc1_b, fc2_w, fc2_b)
    res = run_bass_kernel_spmd(nc, in_maps, core_ids=list(range(N_CORES)),
                               trace=TRACE)
    LAST_EXEC_NS = res.exec_time_ns
    total = math.fsum(float(r["out"][0, 0]) for r in res.results)
    return np.float32(total / N)


# revision 28
# speedup vs baseline: 1.0326x; 1.0094x over previous
"""GRACE contrastive loss kernel for Trainium2 (8 NeuronCores, SPMD).

Strategy (row-block data parallel + symmetry harvesting):
  - Shard the N=8192 nodes across 8 cores (NB=1024 rows each).  Each core
    projects its z1/z2 block through the 2-layer MLP (fp16 matmuls, fp32
    accum), row-normalizes (DVE bit-trick rsqrt, no act-table switch), and
    AllGathers the normalized embeddings (fp16).
  - Similarity work per core is 18 block-units of [1024 x 1024] x K=512
    (vs 32 for the naive 4-matrix scheme):
      * S12 (between_sim): all 8 column blocks — row sums via exp accum_out;
        column sums (ones-matmuls over the exp tiles) give the row sums of
        S21 = S12^T, so S21 is never materialized.
      * S11/S22 (refl_sims): diagonal block locally (no gather needed, runs
        under the AllGathers), plus a shift-invariant triangular assignment:
        unit u in {1,2,3} -> S11 block (c+u)%8, u in {5,6,7} -> S22 block
        (c+u)%8, and u=4 computed by BOTH pair members for BOTH matrices
        with exp pre-halved via bias=-ln(2).  Column sums of each
        off-diagonal exp tile are the transposed block's row-sum
        contribution.
  - All cross-core terms meet in one ReduceScatter over [slot(8) x class(3)
    x 1024] f32; slot c returns exactly core c's total S11/S22/S21 row sums.
    Core-dependent gather/scatter positions use dynamic (register) offsets
    read from tiny per-core uint32 index tables.
"""

import math
import sys

import numpy as np

sys.path.insert(0, "/opt/trn_rl_repo")

import concourse.bass as bass  # noqa: E402
import concourse.mybir as mybir  # noqa: E402
import concourse.tile as tile  # noqa: E402
from concourse import bacc  # noqa: E402
from concourse.bass_utils import run_bass_kernel_spmd  # noqa: E402

F32 = mybir.dt.float32
F16 = mybir.dt.float16
F8 = mybir.dt.float8e4
U32 = mybir.dt.uint32
DR = mybir.MatmulPerfMode.DoubleRow
AF = mybir.ActivationFunctionType
ALU = mybir.AluOpType
SP = mybir.EngineType.SP
PE = mybir.EngineType.PE

N_CORES = 8
N = 8192
D = 512            # feature dim (= H = P in the reference MLP)
NB = N // N_CORES  # 1024 rows per core
KT = D // 128      # 4 k-subtiles
KT2 = KT // 2      # 2 DoubleRow k-subtiles (K=256 each)
F8SCALE = 16.0     # embeddings are shipped as fp8e4 * F8SCALE
MT = NB // 128     # 8 row tiles per core
NCHUNK = 512       # matmul moving free dim (one PSUM bank)
TAU_INV = 2.0      # 1 / tau
E2 = float(np.exp(2.0, dtype=np.float64))  # exp(diag(refl_sim)/tau)
LN2 = float(np.log(2.0))
RSQRT_MAGIC = 0x5F3759DF

TRACE = False
DEBUG = False
LAST_EXEC_NS = None
_CACHE = {}


def _build_program():
    nc = bacc.Bacc("TRN2", target_bir_lowering=False, debug=False,
                   num_devices=N_CORES)

    # ---- I/O ----
    zt1 = nc.dram_tensor("zt1", [128, KT, NB], F16, kind="ExternalInput").ap()
    zt2 = nc.dram_tensor("zt2", [128, KT, NB], F16, kind="ExternalInput").ap()
    w1t = nc.dram_tensor("w1t", [128, KT, D], F16, kind="ExternalInput").ap()
    w2t = nc.dram_tensor("w2t", [128, KT, D], F16, kind="ExternalInput").ap()
    b1 = nc.dram_tensor("b1", [128, KT], F32, kind="ExternalInput").ap()
    b2 = nc.dram_tensor("b2", [128, KT], F32, kind="ExternalInput").ap()
    # per-core index tables (uint32):
    #   gidx[0, u] u=1..4 : gather block (c+u)%8 in the n1 half
    #   gidx[1, u] u=4..7 : gather block (c+u)%8 in the n2 half
    #   gidx[2, 0]        : my slot c
    gidx = nc.dram_tensor("gidx", [3, 8], U32, kind="ExternalInput").ap()
    ident = nc.dram_tensor("ident", [128, 128], F32, kind="ExternalInput").ap()
    out = nc.dram_tensor("out", [1, 1], F32, kind="ExternalOutput").ap()
    if DEBUG:
        dbg_rn = nc.dram_tensor("dbg_rn", [2, NB], F32,
                                kind="ExternalOutput").ap()
        dbg_rs = nc.dram_tensor("dbg_rs", [128, 3 * MT], F32,
                                kind="ExternalOutput").ap()
        dbg_d = nc.dram_tensor("dbg_d", [128, 4 * MT], F32,
                               kind="ExternalOutput").ap()

    rg = [list(range(N_CORES))]

    with tile.TileContext(nc) as tc:
        with tc.tile_pool(name="persist", bufs=1) as persist, \
             tc.tile_pool(name="dram", bufs=1, space="DRAM") as dram:

            ones_col = persist.tile([128, 1], F32)
            nc.vector.memset(ones_col[:], 1.0)
            ones_h = persist.tile([128, 1], F16)
            nc.vector.memset(ones_h[:], 1.0)
            ones_row = persist.tile([1, 128], F32)
            nc.vector.memset(ones_row[:], F8SCALE)
            negln2 = persist.tile([128, 1], F32)
            nc.vector.memset(negln2[:], -LN2)
            id_sb = persist.tile([128, 128], F32, name="id_sb")
            nc.sync.dma_start(id_sb[:], ident)

            # local normalized blocks [feature, node] fp8 * F8SCALE;
            # DoubleRow reads kt-pairs as strided [Ki, Ko=2, *] planes
            n_loc = [persist.tile([128, KT, NB], F8, name=f"n{v}_loc")
                     for v in range(2)]
            rn_vec = [persist.tile([1, NB], F32, name=f"rn{v}") for v in range(2)]

            # ship layout = the SBUF n_loc layout, flattened: [128, 4096] f8
            cc_in = [dram.tile([128, KT * NB], F8, name=f"cc_in{v}")
                     for v in range(2)]
            cc_out = [dram.tile([N_CORES * 128, KT * NB], F8,
                                name=f"cc_out{v}", addr_space="Shared",
                                tag=("agbuf0" if v == 0 else "agbuf1"))
                      for v in range(2)]

            # ReduceScatter staging [slot, class, row] f32;
            # class 0: S11 rowsums, 1: S22 rowsums, 2: S21 rowsums (=colsum S12)
            rs_in = dram.tile([N_CORES, 3, NB], F32, name="rs_in")
            rs_out = dram.tile([3, NB], F32, name="rs_out")

            # rowsum partials from exp accum_out
            parts12 = persist.tile([128, MT, 8], F32, name="parts12")
            parts11 = persist.tile([128, MT, 5], F32, name="parts11")
            parts22 = persist.tile([128, MT, 5], F32, name="parts22")
            pos_sum = persist.tile([1, 1], F32)

            # ---- dynamic index registers ----
            # engines can't read DRAM and the runtime bounds-check assert is
            # broken under this runtime: stage the table in SBUF and skip
            # the runtime check (bounds are guaranteed host-side).
            gidx_sb = persist.tile([1, 24], U32, name="gidx_sb")
            nc.sync.dma_start(gidx_sb[:],
                              gidx[:].rearrange("a b -> (a b)").unsqueeze(0))

            def ld(flat, engines):
                return nc.values_load(gidx_sb[0:1, flat:flat + 1],
                                      engines=engines, min_val=0, max_val=7,
                                      skip_runtime_bounds_check=True)

            g_n1 = {u: ld(u, [SP]) for u in range(1, 5)}
            g_n2 = {u: ld(8 + u, [SP, PE]) for u in range(4, 8)}
            slot_n1 = {u: ld(u, [SP]) for u in range(1, 5)}
            slot_n2 = {u: ld(8 + u, [SP]) for u in range(4, 8)}
            my_slot = ld(16, [SP])
            slot_12 = {k: ld(k if k <= 3 else 8 + k, [SP])
                       for k in range(1, 8)}

            rs_in_r = rs_in[:]  # [8, 3, NB]

            # es + sim-psum pools open for the whole kernel so the
            # diagonal sim units can overlap view-2 projection (PSUM:
            # proj 3+1 banks + sim 4 banks = 8).
            es_pool_cm = tc.tile_pool(name="es", bufs=12)
            sp_pool_cm = tc.tile_pool(name="sim_psum", bufs=2, space="PSUM")
            esp = es_pool_cm.__enter__()
            sp = sp_pool_cm.__enter__()

            # ================= projection phase =================
            with tc.tile_pool(name="proj", bufs=1) as proj, \
                 tc.tile_pool(name="ptmp", bufs=3) as ptmp, \
                 tc.tile_pool(name="ptv", bufs=1) as ptv, \
                 tc.tile_pool(name="ppsum", bufs=3, space="PSUM") as ppsum, \
                 tc.tile_pool(name="spsum", bufs=1, space="PSUM") as spsum:

                # zero the RS staging early (some slots are never written)
                zrow = ptv.tile([1, 3 * NB], F32)
                nc.vector.memset(zrow[:], 0.0)
                for s in range(N_CORES):
                    nc.sync.dma_start(
                        rs_in[s].rearrange("c m -> (c m)").unsqueeze(0),
                        zrow[:])

                zt_sb = [proj.tile([128, KT, NB], F16, name=f"zt{v}_sb")
                         for v in range(2)]
                w1_sb = proj.tile([128, KT, D], F16)
                w2_sb = proj.tile([128, KT, D], F16)
                b1_sb = proj.tile([128, KT], F32)
                b2_sb = proj.tile([128, KT], F32)
                hsq = proj.tile([128, KT, NB], F16)
                hh32 = proj.tile([128, KT, NB], F32)
                e_sb = proj.tile([128, KT, NB], F16)
                h_sb = [proj.tile([128, KT, NB], F32, name=f"h{v}")
                        for v in range(2)]

                nc.sync.dma_start(w1_sb[:], w1t)
                nc.sync.dma_start(zt_sb[0][:], zt1)
                nc.sync.dma_start(b1_sb[:], b1)
                nc.sync.dma_start(w2_sb[:], w2t)
                nc.sync.dma_start(b2_sb[:], b2)
                nc.sync.dma_start(zt_sb[1][:], zt2)

                for v in range(2):
                    # ---- layer 1 + ELU ----
                    for pt in range(KT):
                        for ch in range(NB // NCHUNK):
                            ps = ppsum.tile([128, NCHUNK], F32, tag="ps_proj")
                            for kt in range(KT):
                                nc.tensor.matmul(
                                    ps[:],
                                    lhsT=w1_sb[:, kt, pt * 128:(pt + 1) * 128],
                                    rhs=zt_sb[v][:, kt,
                                                 ch * NCHUNK:(ch + 1) * NCHUNK],
                                    start=(kt == 0), stop=(kt == KT - 1))
                            # elu(y) = relu(y) + min(exp(y),1) - 1, y = ps+b1
                            texp = ptmp.tile([128, NCHUNK], F16, tag="texp")
                            nc.scalar.activation(texp[:], ps[:], AF.Exp,
                                                 bias=b1_sb[:, pt:pt + 1],
                                                 scale=1.0)
                            tmax = ptmp.tile([128, NCHUNK], F16, tag="tmax")
                            nc.scalar.activation(tmax[:], ps[:], AF.Relu,
                                                 bias=b1_sb[:, pt:pt + 1],
                                                 scale=1.0)
                            tclip = ptmp.tile([128, NCHUNK], F16, tag="tclip")
                            nc.vector.tensor_scalar(tclip[:], texp[:], 1.0, -1.0,
                                                    ALU.min, ALU.add)
                            nc.vector.tensor_tensor(
                                e_sb[:, pt, ch * NCHUNK:(ch + 1) * NCHUNK],
                                tmax[:], tclip[:], ALU.add)
                    # ---- layer 2 (+ b2) ----
                    for jt in range(KT):
                        for ch in range(NB // NCHUNK):
                            ps = ppsum.tile([128, NCHUNK], F32, tag="ps_proj")
                            for kt in range(KT):
                                nc.tensor.matmul(
                                    ps[:],
                                    lhsT=w2_sb[:, kt, jt * 128:(jt + 1) * 128],
                                    rhs=e_sb[:, kt, ch * NCHUNK:(ch + 1) * NCHUNK],
                                    start=(kt == 0), stop=(kt == KT - 1))
                            sl = (slice(None), jt,
                                  slice(ch * NCHUNK, (ch + 1) * NCHUNK))
                            nc.scalar.activation(h_sb[v][sl], ps[:], AF.Identity,
                                                 bias=b2_sb[:, jt:jt + 1],
                                                 scale=1.0)
                            nc.vector.tensor_tensor(hsq[sl], h_sb[v][sl],
                                                    h_sb[v][sl], ALU.mult)
                    # ---- ss[i] = sum_f h[f,i]^2 via ones-matmul ----
                    ss = ptv.tile([1, NB], F32, tag="ss")
                    for ch in range(NB // NCHUNK):
                        pss = spsum.tile([1, NCHUNK], F32, tag="ps_small")
                        for jt in range(KT):
                            nc.tensor.matmul(pss[:], lhsT=ones_h[:],
                                             rhs=hsq[:, jt,
                                                     ch * NCHUNK:(ch + 1) * NCHUNK],
                                             start=(jt == 0), stop=(jt == KT - 1))
                        nc.vector.tensor_copy(ss[:, ch * NCHUNK:(ch + 1) * NCHUNK],
                                              pss[:])
                    # rn = rsqrt(ss): bit-trick seed + 1 Newton step (all DVE,
                    # no act-table switch).  y0 = bits(magic - (ss>>1));
                    # rn = y0*(1.5 - 0.5*ss*y0^2)
                    yb = ptv.tile([1, NB], U32, tag="yb")
                    nc.vector.tensor_scalar(yb[:], ss[:].bitcast(U32), -0.5,
                                            float(RSQRT_MAGIC) + 0.5,
                                            ALU.mult, ALU.add)
                    y0 = yb[:].bitcast(F32)
                    t1 = ptv.tile([1, NB], F32, tag="t1")
                    nc.vector.tensor_tensor(t1[:], y0, y0, ALU.mult)
                    nc.vector.tensor_tensor(t1[:], t1[:], ss[:], ALU.mult)
                    nc.vector.tensor_scalar(t1[:], t1[:], -0.5, 1.5,
                                            ALU.mult, ALU.add)
                    nc.vector.tensor_tensor(rn_vec[v][:], y0, t1[:],
                                            ALU.mult)

                    # broadcast rn across partitions (K=1 ones-matmul), scale
                    for ch in range(NB // NCHUNK):
                        pbc = spsum.tile([128, NCHUNK], F32, tag="ps_small")
                        nc.tensor.matmul(
                            pbc[:], lhsT=ones_row[:],
                            rhs=rn_vec[v][:, ch * NCHUNK:(ch + 1) * NCHUNK],
                            start=True, stop=True)
                        for jt in range(KT):
                            sl = (slice(None), jt,
                                  slice(ch * NCHUNK, (ch + 1) * NCHUNK))
                            nc.vector.tensor_tensor(n_loc[v][sl], h_sb[v][sl],
                                                    pbc[:], ALU.mult)

                    # ship to DRAM + AllGather (TOPSP+SDMA, overlaps compute)
                    nc.sync.dma_start(
                        cc_in[v][:],
                        n_loc[v][:].rearrange("p a b -> p (a b)"))
                    nc.gpsimd.collective_compute(
                        "AllGather", ALU.bypass, replica_groups=rg,
                        ins=[cc_in[v].opt()], outs=[cc_out[v].opt()])

                # ---- pos diagonal: s12_ii = rn1_i*rn2_i*sum_f h1[f,i]h2[f,i]
                hh = hh32
                for jt in range(KT):
                    nc.vector.tensor_tensor(hh[:, jt, :], h_sb[0][:, jt, :],
                                            h_sb[1][:, jt, :], ALU.mult)
                pos_part = ptv.tile([1, NB], F32, tag="pos_part")
                for ch in range(NB // NCHUNK):
                    psp = spsum.tile([1, NCHUNK], F32, tag="ps_small")
                    for jt in range(KT):
                        nc.tensor.matmul(psp[:], lhsT=ones_col[:],
                                         rhs=hh[:, jt,
                                                ch * NCHUNK:(ch + 1) * NCHUNK],
                                         start=(jt == 0), stop=(jt == KT - 1))
                    sl = (slice(0, 1), slice(ch * NCHUNK, (ch + 1) * NCHUNK))
                    nc.vector.tensor_tensor(pos_part[sl], psp[:], rn_vec[0][sl],
                                            ALU.mult)
                    nc.vector.tensor_tensor(pos_part[sl], pos_part[sl],
                                            rn_vec[1][sl], ALU.mult)
                nc.vector.tensor_reduce(pos_sum[:], pos_part[:],
                                        mybir.AxisListType.X, ALU.add)

                # ---- diagonal sim units (local data only; hide the AGs) ----
                # emitted inside the proj pool scope so they use proj PSUM
                # budget?  No - they need their own pool; open sim psum pool
                # here is not possible (pool nesting).  Diag units are emitted
                # in the sim section below but depend only on n_loc, so the
                # Tile scheduler can run them as soon as n_loc is ready.

            # ================= similarity phase =================
            # unit list: (key, lhs view, rhs kind, rhs index, bias, parts,
            #             parts slot, colsum dest)
            # rhs kind: 'loc' (n_loc[view]), 'n1' (dyn block of cc_out[0]),
            #           'n2s' (static block j of n2_sb), 'n2d' (dyn block)
            with tc.tile_pool(name="sim", bufs=1) as sim, \
                 tc.tile_pool(name="cs", bufs=6) as csp, \
                 tc.tile_pool(name="csum_psum", bufs=2, space="PSUM") as cp:

                # dyn-loaded gathered blocks (static matmul operands; only
                # the DMA source offsets are dynamic):
                #   rhs1[u] u=1..4: n1 block (c+u)%8   (S11 off-diag units)
                #   rhs2[k] k=1..7: n2 block (c+k)%8   (S12 units; k=4..7
                #                                       shared with S22)
                rhs1 = {u: sim.tile([128, KT, NB], F8, name=f"rhs1_{u}")
                        for u in range(1, 5)}
                rhs2 = {k: sim.tile([128, KT, NB], F8, name=f"rhs2_{k}")
                        for k in range(1, 8)}

                cc0r = cc_out[0][:].rearrange(
                    "(g p) (a m) -> g p a m", p=128, a=KT)
                cc1r = cc_out[1][:].rearrange(
                    "(g p) (a m) -> g p a m", p=128, a=KT)
                for u in range(1, 5):
                    nc.sync.dma_start(rhs1[u][:], cc0r[g_n1[u]])
                for k in range(1, 8):
                    gk = g_n1[k] if k <= 3 else g_n2[k]
                    nc.sync.dma_start(rhs2[k][:], cc1r[gk])

                # pending colsum pass emitters, delayed by one unit
                pending = []

                def emit_pending():
                    while pending:
                        pending.pop(0)()

                def sim_unit(lhs, rhs_fn, bias, parts, pslot, colsum_dst,
                             keep_es, act_accum=False):
                    """One [NB x NB] unit: 8 row tiles x (4 DR MMs + exp ACT).
                    rhs_fn(kt2, c0, c1) -> AP of [128, c1-c0, 2] moving pairs.
                    colsum_dst: None or a dram AP receiving [1, NB]."""
                    es_tiles = []
                    for mt in range(MT):
                        ps = sp.tile([128, NB], F32, tag="ps_sim")
                        for kt2 in range(KT2):
                            for ck in range(2):
                                nc.tensor.matmul(
                                    ps[:, ck * NCHUNK:(ck + 1) * NCHUNK],
                                    lhsT=lhs[:, 2 * kt2:2 * kt2 + 2,
                                             mt * 128:(mt + 1) * 128],
                                    rhs=rhs_fn(kt2, ck * NCHUNK,
                                               (ck + 1) * NCHUNK),
                                    start=(kt2 == 0), stop=(kt2 == KT2 - 1),
                                    perf_mode=DR)
                        es = esp.tile([128, NB], F16, tag="es")
                        if keep_es:
                            es_tiles.append(es)
                        if act_accum:
                            nc.scalar.activation(
                                es[:], ps[:], AF.Exp,
                                scale=TAU_INV / (F8SCALE * F8SCALE), bias=bias,
                                accum_out=parts[:, mt, pslot:pslot + 1])
                        else:
                            nc.scalar.activation(
                                es[:], ps[:], AF.Exp,
                                scale=TAU_INV / (F8SCALE * F8SCALE), bias=bias)
                            nc.vector.tensor_reduce(
                                parts[:, mt, pslot:pslot + 1], es[:],
                                mybir.AxisListType.X, ALU.add)
                        # emit the previous unit's colsum pass midway
                        if mt == 3:
                            emit_pending()

                    if colsum_dst is not None:
                        def colsum_pass(es_tiles=es_tiles,
                                        colsum_dst=colsum_dst):
                            cs = csp.tile([1, NB], F32, tag="cs")
                            for ck in range(2):
                                psc = cp.tile([1, NCHUNK], F32, tag="psc")
                                for mt in range(MT):
                                    nc.tensor.matmul(
                                        psc[:],
                                        lhsT=ones_h[:],
                                        rhs=es_tiles[mt][:,
                                                         ck * NCHUNK:
                                                         (ck + 1) * NCHUNK],
                                        start=(mt == 0), stop=(mt == MT - 1),
                                        skip_group_check=True)
                                nc.vector.tensor_copy(
                                    cs[:, ck * NCHUNK:(ck + 1) * NCHUNK],
                                    psc[:])
                            nc.sync.dma_start(colsum_dst, cs[:])
                        pending.append(colsum_pass)

                def loc_rhs(v):
                    return lambda kt2, c0, c1: \
                        n_loc[v][:, 2 * kt2:2 * kt2 + 2, c0:c1]

                def rhs1_rhs(u):
                    return lambda kt2, c0, c1: \
                        rhs1[u][:, 2 * kt2:2 * kt2 + 2, c0:c1]

                def n2s_rhs(k):
                    return lambda kt2, c0, c1: \
                        rhs2[k][:, 2 * kt2:2 * kt2 + 2, c0:c1]

                n2d_rhs = n2s_rhs

                # --- diagonal units (only need n_loc; run under the AGs) ---
                sim_unit(n_loc[0], loc_rhs(0), 0.0, parts11, 0, None,
                         False, act_accum=True)
                sim_unit(n_loc[1], loc_rhs(1), 0.0, parts22, 0, None,
                         False, act_accum=True)
                # --- S12 diagonal block: rhs is the local n2 ---
                dst = rs_in_r[my_slot, 2, :].unsqueeze(0)
                sim_unit(n_loc[0], loc_rhs(1), 0.0, parts12, 0, dst, True)

                # --- S11 off-diagonal units u=1..4 (need AG1) ---
                for u in range(1, 5):
                    bias = negln2 if u == 4 else 0.0
                    dst = rs_in_r[slot_n1[u], 0, :].unsqueeze(0)
                    sim_unit(n_loc[0], rhs1_rhs(u), bias, parts11, u, dst,
                             True)

                # --- S12 off-diagonal units k=1..7 (need AG2) ---
                for k in range(1, 8):
                    dst = rs_in_r[slot_12[k], 2, :].unsqueeze(0)
                    sim_unit(n_loc[0], n2s_rhs(k), 0.0, parts12, k, dst, True)

                # --- S22 off-diagonal units u=4..7 (need AG2) ---
                for u in range(4, 8):
                    bias = negln2 if u == 4 else 0.0
                    dst = rs_in_r[slot_n2[u], 1, :].unsqueeze(0)
                    sim_unit(n_loc[1], n2d_rhs(u), bias, parts22,
                             u - 3, dst, True)

                emit_pending()

                # --- my-rows rowsum partials stay LOCAL (only colsums
                # need the cross-core reduce) ---
                my11 = sim.tile([128, MT], F32)
                my22 = sim.tile([128, MT], F32)
                rs12l = sim.tile([128, MT], F32)
                nc.vector.tensor_reduce(my11[:], parts11[:],
                                        mybir.AxisListType.X, ALU.add)
                nc.vector.tensor_reduce(my22[:], parts22[:],
                                        mybir.AxisListType.X, ALU.add)
                nc.vector.tensor_reduce(rs12l[:], parts12[:],
                                        mybir.AxisListType.X, ALU.add)

                if DEBUG:
                    for v in range(2):
                        nc.sync.dma_start(dbg_rn[v:v + 1, :], rn_vec[v][:])

                # dummy Ln: pulls the ln act-table load into the RS window
                dln = sim.tile([1, 1], F32)
                nc.scalar.activation(dln[:], ones_col[0:1, 0:1], AF.Ln)

                # --- ReduceScatter: slot c -> colsum contributions for my
                # rows (class 0: S11, 1: S22, 2: S21) ---
                nc.gpsimd.collective_compute(
                    "ReduceScatter", ALU.add, replica_groups=rg,
                    ins=[rs_in.opt()], outs=[rs_out.opt()])

                # --- final assembly ---
                # contiguous load [24, 128] then transpose via identity matmul
                rsC = sim.tile([24, 128], F32)
                nc.sync.dma_start(
                    rsC[:], rs_out[:].rearrange("c (mt p) -> (c mt) p", p=128))
                d12 = sim.tile([128, 2 * MT], F32)
                d1 = d12[:, 0:MT]
                d2 = d12[:, MT:2 * MT]
                with tc.tile_pool(name="fin_psum", bufs=1,
                                  space="PSUM") as fp:
                    ptr = fp.tile([128, 24], F32)
                    nc.tensor.matmul(ptr[:], lhsT=rsC[:],
                                     rhs=id_sb[0:24, 0:24],
                                     start=True, stop=True)
                    # totals (for debug + denominators)
                    t0 = sim.tile([128, MT], F32)
                    t1d = sim.tile([128, MT], F32)
                    nc.vector.tensor_tensor(t0[:], ptr[:, 0:MT], my11[:],
                                            ALU.add)
                    nc.vector.tensor_tensor(t1d[:], ptr[:, MT:2 * MT],
                                            my22[:], ALU.add)
                    nc.vector.tensor_tensor(d1, t0[:], rs12l[:], ALU.add)
                    nc.vector.tensor_tensor(d2, t1d[:],
                                            ptr[:, 2 * MT:3 * MT], ALU.add)
                    nc.vector.tensor_scalar_add(d12[:], d12[:], -E2)
                    if DEBUG:
                        t2 = sim.tile([128, MT], F32)
                        nc.vector.tensor_copy(t2[:], ptr[:, 2 * MT:3 * MT])
                        nc.sync.dma_start(dbg_rs[:, 0:MT], t0[:])
                        nc.sync.dma_start(dbg_rs[:, MT:2 * MT], t1d[:])
                        nc.sync.dma_start(dbg_rs[:, 2 * MT:3 * MT], t2[:])
                        nc.sync.dma_start(dbg_d[:, 0:2 * MT], d12[:])
                        nc.sync.dma_start(dbg_d[:, 2 * MT:3 * MT], rs12l[:])
                        nc.sync.dma_start(dbg_d[:, 3 * MT:4 * MT], my11[:])
                    nc.scalar.activation(d12[:], d12[:], AF.Ln)
                    lsum = sim.tile([128, MT], F32)
                    nc.vector.tensor_tensor(lsum[:], d1, d2, ALU.add)
                    lrow = sim.tile([128, 1], F32)
                    nc.vector.tensor_reduce(lrow[:], lsum[:],
                                            mybir.AxisListType.X, ALU.add)
                    pfin = fp.tile([1, 1], F32)
                    nc.tensor.matmul(pfin[:], lhsT=ones_col[:], rhs=lrow[:],
                                     start=True, stop=True)
                    fin = sim.tile([1, 1], F32)
                    nc.vector.tensor_scalar_mul(fin[:], pfin[:], 0.5)
                    p2 = sim.tile([1, 1], F32)
                    nc.vector.tensor_scalar_mul(p2[:], pos_sum[:], 2.0)
                    nc.vector.tensor_tensor(fin[:], fin[:], p2[:],
                                            ALU.subtract)
                    nc.sync.dma_start(out, fin[:])

            sp_pool_cm.__exit__(None, None, None)
            es_pool_cm.__exit__(None, None, None)

    nc.compile()
    return nc


def _prep_inputs(z1, z2, fc1_w, fc1_b, fc2_w, fc2_b):
    """Host-side shard + layout prep. Returns in_maps for the 8 cores."""
    w1t = np.ascontiguousarray(fc1_w.T).reshape(KT, 128, D).transpose(1, 0, 2)
    w1t = np.ascontiguousarray(w1t, dtype=np.float16)
    w2t = np.ascontiguousarray(fc2_w.T).reshape(KT, 128, D).transpose(1, 0, 2)
    w2t = np.ascontiguousarray(w2t, dtype=np.float16)
    b1 = np.ascontiguousarray(fc1_b.reshape(KT, 128).T, dtype=np.float32)
    b2 = np.ascontiguousarray(fc2_b.reshape(KT, 128).T, dtype=np.float32)

    in_maps = []
    for c in range(N_CORES):
        blk1 = z1[c * NB:(c + 1) * NB].T            # [512, 1024]
        blk2 = z2[c * NB:(c + 1) * NB].T
        zt1 = np.ascontiguousarray(
            blk1.reshape(KT, 128, NB).transpose(1, 0, 2), dtype=np.float16)
        zt2 = np.ascontiguousarray(
            blk2.reshape(KT, 128, NB).transpose(1, 0, 2), dtype=np.float16)
        gidx = np.zeros((3, 8), np.uint32)
        for u in range(1, 5):
            gidx[0, u] = (c + u) % N_CORES
        for u in range(4, 8):
            gidx[1, u] = (c + u) % N_CORES
        gidx[2, 0] = c
        in_maps.append({"zt1": zt1, "zt2": zt2, "w1t": w1t, "w2t": w2t,
                        "b1": b1, "b2": b2, "gidx": gidx,
                        "ident": np.eye(128, dtype=np.float32)})
    return in_maps


def kernel(z1, z2, fc1_w, fc1_b, fc2_w, fc2_b):
    global LAST_EXEC_NS
    z1 = np.asarray(z1, dtype=np.float32)
    z2 = np.asarray(z2, dtype=np.float32)
    fc1_w = np.asarray(fc1_w, dtype=np.float32)
    fc1_b = np.asarray(fc1_b, dtype=np.float32)
    fc2_w = np.asarray(fc2_w, dtype=np.float32)
    fc2_b = np.asarray(fc2_b, dtype=np.float32)

    if "nc" not in _CACHE:
        _CACHE["nc"] = _build_program()
    nc = _CACHE["nc"]

    in_maps = _prep_inputs(z1, z2, fc1_w, fc1_b, fc2_w, fc2_b)
    res = run_bass_kernel_spmd(nc, in_maps, core_ids=list(range(N_CORES)),
                               trace=TRACE)
    LAST_EXEC_NS = res.exec_time_ns
    total = math.fsum(float(r["out"][0, 0]) for r in res.results)
    return np.float32(total / N)


# revision 30
# speedup vs baseline: 1.2115x; 1.1733x over previous
"""GRACE contrastive loss kernel for Trainium2 (8 NeuronCores, SPMD).

Strategy (row-block data parallel + symmetry harvesting):
  - Shard the N=8192 nodes across 8 cores (NB=1024 rows each).  Each core
    projects its z1/z2 block through the 2-layer MLP (fp16 matmuls, fp32
    accum), row-normalizes (DVE bit-trick rsqrt, no act-table switch), and
    AllGathers the normalized embeddings (fp16).
  - Similarity work per core is 18 block-units of [1024 x 1024] x K=512
    (vs 32 for the naive 4-matrix scheme):
      * S12 (between_sim): all 8 column blocks — row sums via exp accum_out;
        column sums (ones-matmuls over the exp tiles) give the row sums of
        S21 = S12^T, so S21 is never materialized.
      * S11/S22 (refl_sims): diagonal block locally (no gather needed, runs
        under the AllGathers), plus a shift-invariant triangular assignment:
        unit u in {1,2,3} -> S11 block (c+u)%8, u in {5,6,7} -> S22 block
        (c+u)%8, and u=4 computed by BOTH pair members for BOTH matrices
        with exp pre-halved via bias=-ln(2).  Column sums of each
        off-diagonal exp tile are the transposed block's row-sum
        contribution.
  - All cross-core terms meet in one ReduceScatter over [slot(8) x class(3)
    x 1024] f32; slot c returns exactly core c's total S11/S22/S21 row sums.
    Core-dependent gather/scatter positions use dynamic (register) offsets
    read from tiny per-core uint32 index tables.
"""

import math
import sys

import numpy as np

sys.path.insert(0, "/opt/trn_rl_repo")

import concourse.bass as bass  # noqa: E402
import concourse.mybir as mybir  # noqa: E402
import concourse.tile as tile  # noqa: E402
from concourse import bacc  # noqa: E402
from concourse.bass_utils import run_bass_kernel_spmd  # noqa: E402

F32 = mybir.dt.float32
F16 = mybir.dt.float16
F8 = mybir.dt.float8e4
U32 = mybir.dt.uint32
DR = mybir.MatmulPerfMode.DoubleRow
AF = mybir.ActivationFunctionType
ALU = mybir.AluOpType
SP = mybir.EngineType.SP
PE = mybir.EngineType.PE

N_CORES = 8
N = 8192
D = 512            # feature dim (= H = P in the reference MLP)
NB = N // N_CORES  # 1024 rows per core
KT = D // 128      # 4 k-subtiles
KT2 = KT // 2      # 2 DoubleRow k-subtiles (K=256 each)
F8SCALE = 16.0     # embeddings are shipped as fp8e4 * F8SCALE
MT = NB // 128     # 8 row tiles per core
NCHUNK = 512       # matmul moving free dim (one PSUM bank)
TAU_INV = 2.0      # 1 / tau
E2 = float(np.exp(2.0, dtype=np.float64))  # exp(diag(refl_sim)/tau)
LN2 = float(np.log(2.0))
RSQRT_MAGIC = 0x5F3759DF

TRACE = False
DEBUG = False
LAST_EXEC_NS = None
_CACHE = {}


def _build_program():
    nc = bacc.Bacc("TRN2", target_bir_lowering=False, debug=False,
                   num_devices=N_CORES)

    # ---- I/O ----
    zt1 = nc.dram_tensor("zt1", [128, KT, NB], F16, kind="ExternalInput").ap()
    zt2 = nc.dram_tensor("zt2", [128, KT, NB], F16, kind="ExternalInput").ap()
    w1t = nc.dram_tensor("w1t", [128, KT, D], F16, kind="ExternalInput").ap()
    w2t = nc.dram_tensor("w2t", [128, KT, D], F16, kind="ExternalInput").ap()
    b1 = nc.dram_tensor("b1", [128, KT], F32, kind="ExternalInput").ap()
    b2 = nc.dram_tensor("b2", [128, KT], F32, kind="ExternalInput").ap()
    # per-core index tables (uint32):
    #   gidx[0, u] u=1..4 : gather block (c+u)%8 in the n1 half
    #   gidx[1, u] u=4..7 : gather block (c+u)%8 in the n2 half
    #   gidx[2, 0]        : my slot c
    gidx = nc.dram_tensor("gidx", [3, 8], U32, kind="ExternalInput").ap()
    ident = nc.dram_tensor("ident", [128, 128], F32, kind="ExternalInput").ap()
    out = nc.dram_tensor("out", [1, 1], F32, kind="ExternalOutput").ap()
    if DEBUG:
        dbg_rn = nc.dram_tensor("dbg_rn", [2, NB], F32,
                                kind="ExternalOutput").ap()
        dbg_rs = nc.dram_tensor("dbg_rs", [128, 3 * MT], F32,
                                kind="ExternalOutput").ap()
        dbg_d = nc.dram_tensor("dbg_d", [128, 4 * MT], F32,
                               kind="ExternalOutput").ap()

    rg = [list(range(N_CORES))]

    with tile.TileContext(nc) as tc:
        with tc.tile_pool(name="persist", bufs=1) as persist, \
             tc.tile_pool(name="dram", bufs=1, space="DRAM") as dram:

            ones_col = persist.tile([128, 1], F32)
            nc.vector.memset(ones_col[:], 1.0)
            ones_h = persist.tile([128, 1], F16)
            nc.vector.memset(ones_h[:], 1.0)
            ones_dr = persist.tile([128, 2, 16], F8)
            nc.vector.memset(ones_dr[:], 1.0)
            ones_row = persist.tile([1, 128], F32)
            nc.vector.memset(ones_row[:], F8SCALE)
            negln2 = persist.tile([128, 1], F32)
            nc.vector.memset(negln2[:], -LN2)
            id_sb = persist.tile([128, 128], F32, name="id_sb")
            nc.sync.dma_start(id_sb[:], ident)

            # local normalized blocks [feature, node] fp8 * F8SCALE;
            # DoubleRow reads kt-pairs as strided [Ki, Ko=2, *] planes
            n_loc = [persist.tile([128, KT, NB], F8, name=f"n{v}_loc")
                     for v in range(2)]
            rn_vec = [persist.tile([1, NB], F32, name=f"rn{v}") for v in range(2)]

            # ship layout = the SBUF n_loc layout, flattened: [128, 4096] f8
            cc_in = [dram.tile([128, KT * NB], F8, name=f"cc_in{v}")
                     for v in range(2)]
            cc_out = [dram.tile([N_CORES * 128, KT * NB], F8,
                                name=f"cc_out{v}", addr_space="Shared",
                                tag=("agbuf0" if v == 0 else "agbuf1"))
                      for v in range(2)]

            # ReduceScatter staging [slot, class, row] f32;
            # class 0: S11 rowsums, 1: S22 rowsums, 2: S21 rowsums (=colsum S12)
            rs_in = dram.tile([N_CORES, 3, NB], F32, name="rs_in")
            rs_out = dram.tile([3, NB], F32, name="rs_out")

            # rowsum partials from exp accum_out
            parts12 = persist.tile([128, MT, 8], F32, name="parts12")
            parts11 = persist.tile([128, MT, 5], F32, name="parts11")
            parts22 = persist.tile([128, MT, 5], F32, name="parts22")
            pos_sum = persist.tile([1, 1], F32)

            # ---- dynamic index registers ----
            # engines can't read DRAM and the runtime bounds-check assert is
            # broken under this runtime: stage the table in SBUF and skip
            # the runtime check (bounds are guaranteed host-side).
            gidx_sb = persist.tile([1, 24], U32, name="gidx_sb")
            nc.sync.dma_start(gidx_sb[:],
                              gidx[:].rearrange("a b -> (a b)").unsqueeze(0))

            def ld(flat, engines):
                return nc.values_load(gidx_sb[0:1, flat:flat + 1],
                                      engines=engines, min_val=0, max_val=7,
                                      skip_runtime_bounds_check=True)

            g_n1 = {u: ld(u, [SP]) for u in range(1, 5)}
            g_n2 = {u: ld(8 + u, [SP, PE]) for u in range(4, 8)}
            slot_n1 = {u: ld(u, [SP]) for u in range(1, 5)}
            slot_n2 = {u: ld(8 + u, [SP]) for u in range(4, 8)}
            my_slot = ld(16, [SP])
            slot_12 = {k: ld(k if k <= 3 else 8 + k, [SP])
                       for k in range(1, 8)}

            rs_in_r = rs_in[:]  # [8, 3, NB]

            # es + sim-psum pools open for the whole kernel so the
            # diagonal sim units can overlap view-2 projection (PSUM:
            # proj 3+1 banks + sim 4 banks = 8).
            es_pool_cm = tc.tile_pool(name="es", bufs=12)
            sp_pool_cm = tc.tile_pool(name="sim_psum", bufs=2, space="PSUM")
            esp = es_pool_cm.__enter__()
            sp = sp_pool_cm.__enter__()

            # ================= projection phase =================
            with tc.tile_pool(name="proj", bufs=1) as proj, \
                 tc.tile_pool(name="ptmp", bufs=3) as ptmp, \
                 tc.tile_pool(name="ptv", bufs=1) as ptv, \
                 tc.tile_pool(name="ppsum", bufs=3, space="PSUM") as ppsum, \
                 tc.tile_pool(name="spsum", bufs=1, space="PSUM") as spsum:

                # zero the RS staging early (some slots are never written)
                zrow = ptv.tile([1, 3 * NB], F32)
                nc.vector.memset(zrow[:], 0.0)
                for s in range(N_CORES):
                    nc.sync.dma_start(
                        rs_in[s].rearrange("c m -> (c m)").unsqueeze(0),
                        zrow[:])

                zt_sb = [proj.tile([128, KT, NB], F16, name=f"zt{v}_sb")
                         for v in range(2)]
                w1_sb = proj.tile([128, KT, D], F16)
                w2_sb = proj.tile([128, KT, D], F16)
                b1_sb = proj.tile([128, KT], F32)
                b2_sb = proj.tile([128, KT], F32)
                hsq = proj.tile([128, KT, NB], F16)
                hh32 = proj.tile([128, KT, NB], F32)
                e_sb = proj.tile([128, KT, NB], F16)
                h_sb = [proj.tile([128, KT, NB], F32, name=f"h{v}")
                        for v in range(2)]

                nc.sync.dma_start(w1_sb[:], w1t)
                nc.sync.dma_start(zt_sb[0][:], zt1)
                nc.sync.dma_start(b1_sb[:], b1)
                nc.sync.dma_start(w2_sb[:], w2t)
                nc.sync.dma_start(b2_sb[:], b2)
                nc.sync.dma_start(zt_sb[1][:], zt2)

                for v in range(2):
                    # ---- layer 1 + ELU ----
                    for pt in range(KT):
                        for ch in range(NB // NCHUNK):
                            ps = ppsum.tile([128, NCHUNK], F32, tag="ps_proj")
                            for kt in range(KT):
                                nc.tensor.matmul(
                                    ps[:],
                                    lhsT=w1_sb[:, kt, pt * 128:(pt + 1) * 128],
                                    rhs=zt_sb[v][:, kt,
                                                 ch * NCHUNK:(ch + 1) * NCHUNK],
                                    start=(kt == 0), stop=(kt == KT - 1))
                            # elu(y) = relu(y) + min(exp(y),1) - 1, y = ps+b1
                            texp = ptmp.tile([128, NCHUNK], F16, tag="texp")
                            nc.scalar.activation(texp[:], ps[:], AF.Exp,
                                                 bias=b1_sb[:, pt:pt + 1],
                                                 scale=1.0)
                            tmax = ptmp.tile([128, NCHUNK], F16, tag="tmax")
                            nc.scalar.activation(tmax[:], ps[:], AF.Relu,
                                                 bias=b1_sb[:, pt:pt + 1],
                                                 scale=1.0)
                            tclip = ptmp.tile([128, NCHUNK], F16, tag="tclip")
                            nc.vector.tensor_scalar(tclip[:], texp[:], 1.0, -1.0,
                                                    ALU.min, ALU.add)
                            nc.vector.tensor_tensor(
                                e_sb[:, pt, ch * NCHUNK:(ch + 1) * NCHUNK],
                                tmax[:], tclip[:], ALU.add)
                    # ---- layer 2 (+ b2) ----
                    for jt in range(KT):
                        for ch in range(NB // NCHUNK):
                            ps = ppsum.tile([128, NCHUNK], F32, tag="ps_proj")
                            for kt in range(KT):
                                nc.tensor.matmul(
                                    ps[:],
                                    lhsT=w2_sb[:, kt, jt * 128:(jt + 1) * 128],
                                    rhs=e_sb[:, kt, ch * NCHUNK:(ch + 1) * NCHUNK],
                                    start=(kt == 0), stop=(kt == KT - 1))
                            sl = (slice(None), jt,
                                  slice(ch * NCHUNK, (ch + 1) * NCHUNK))
                            nc.scalar.activation(h_sb[v][sl], ps[:], AF.Identity,
                                                 bias=b2_sb[:, jt:jt + 1],
                                                 scale=1.0)
                            nc.vector.tensor_tensor(hsq[sl], h_sb[v][sl],
                                                    h_sb[v][sl], ALU.mult)
                    # ---- ss[i] = sum_f h[f,i]^2 via ones-matmul ----
                    ss = ptv.tile([1, NB], F32, tag="ss")
                    for ch in range(NB // NCHUNK):
                        pss = spsum.tile([1, NCHUNK], F32, tag="ps_small")
                        for jt in range(KT):
                            nc.tensor.matmul(pss[:], lhsT=ones_h[:],
                                             rhs=hsq[:, jt,
                                                     ch * NCHUNK:(ch + 1) * NCHUNK],
                                             start=(jt == 0), stop=(jt == KT - 1))
                        nc.vector.tensor_copy(ss[:, ch * NCHUNK:(ch + 1) * NCHUNK],
                                              pss[:])
                    # rn = rsqrt(ss): bit-trick seed + 1 Newton step (all DVE,
                    # no act-table switch).  y0 = bits(magic - (ss>>1));
                    # rn = y0*(1.5 - 0.5*ss*y0^2)
                    yb = ptv.tile([1, NB], U32, tag="yb")
                    nc.vector.tensor_scalar(yb[:], ss[:].bitcast(U32), -0.5,
                                            float(RSQRT_MAGIC) + 0.5,
                                            ALU.mult, ALU.add)
                    y0 = yb[:].bitcast(F32)
                    t1 = ptv.tile([1, NB], F32, tag="t1")
                    nc.vector.tensor_tensor(t1[:], y0, y0, ALU.mult)
                    nc.vector.tensor_tensor(t1[:], t1[:], ss[:], ALU.mult)
                    nc.vector.tensor_scalar(t1[:], t1[:], -0.5, 1.5,
                                            ALU.mult, ALU.add)
                    nc.vector.tensor_tensor(rn_vec[v][:], y0, t1[:],
                                            ALU.mult)

                    # broadcast rn across partitions (K=1 ones-matmul), scale
                    for ch in range(NB // NCHUNK):
                        pbc = spsum.tile([128, NCHUNK], F32, tag="ps_small")
                        nc.tensor.matmul(
                            pbc[:], lhsT=ones_row[:],
                            rhs=rn_vec[v][:, ch * NCHUNK:(ch + 1) * NCHUNK],
                            start=True, stop=True)
                        for jt in range(KT):
                            sl = (slice(None), jt,
                                  slice(ch * NCHUNK, (ch + 1) * NCHUNK))
                            nc.vector.tensor_tensor(n_loc[v][sl], h_sb[v][sl],
                                                    pbc[:], ALU.mult)

                    # ship to DRAM + AllGather (TOPSP+SDMA, overlaps compute)
                    nc.sync.dma_start(
                        cc_in[v][:],
                        n_loc[v][:].rearrange("p a b -> p (a b)"))
                    nc.gpsimd.collective_compute(
                        "AllGather", ALU.bypass, replica_groups=rg,
                        ins=[cc_in[v].opt()], outs=[cc_out[v].opt()])

                # ---- pos diagonal: s12_ii = rn1_i*rn2_i*sum_f h1[f,i]h2[f,i]
                hh = hh32
                for jt in range(KT):
                    nc.vector.tensor_tensor(hh[:, jt, :], h_sb[0][:, jt, :],
                                            h_sb[1][:, jt, :], ALU.mult)
                pos_part = ptv.tile([1, NB], F32, tag="pos_part")
                for ch in range(NB // NCHUNK):
                    psp = spsum.tile([1, NCHUNK], F32, tag="ps_small")
                    for jt in range(KT):
                        nc.tensor.matmul(psp[:], lhsT=ones_col[:],
                                         rhs=hh[:, jt,
                                                ch * NCHUNK:(ch + 1) * NCHUNK],
                                         start=(jt == 0), stop=(jt == KT - 1))
                    sl = (slice(0, 1), slice(ch * NCHUNK, (ch + 1) * NCHUNK))
                    nc.vector.tensor_tensor(pos_part[sl], psp[:], rn_vec[0][sl],
                                            ALU.mult)
                    nc.vector.tensor_tensor(pos_part[sl], pos_part[sl],
                                            rn_vec[1][sl], ALU.mult)
                nc.vector.tensor_reduce(pos_sum[:], pos_part[:],
                                        mybir.AxisListType.X, ALU.add)

                # ---- diagonal sim units (local data only; hide the AGs) ----
                # emitted inside the proj pool scope so they use proj PSUM
                # budget?  No - they need their own pool; open sim psum pool
                # here is not possible (pool nesting).  Diag units are emitted
                # in the sim section below but depend only on n_loc, so the
                # Tile scheduler can run them as soon as n_loc is ready.

            # ================= similarity phase =================
            # unit list: (key, lhs view, rhs kind, rhs index, bias, parts,
            #             parts slot, colsum dest)
            # rhs kind: 'loc' (n_loc[view]), 'n1' (dyn block of cc_out[0]),
            #           'n2s' (static block j of n2_sb), 'n2d' (dyn block)
            with tc.tile_pool(name="sim", bufs=1) as sim, \
                 tc.tile_pool(name="cs", bufs=6) as csp, \
                 tc.tile_pool(name="csum_psum", bufs=2, space="PSUM") as cp:

                # dyn-loaded gathered blocks (static matmul operands; only
                # the DMA source offsets are dynamic):
                #   rhs1[u] u=1..4: n1 block (c+u)%8   (S11 off-diag units)
                #   rhs2[k] k=1..7: n2 block (c+k)%8   (S12 units; k=4..7
                #                                       shared with S22)
                rhs1 = {u: sim.tile([128, KT, NB], F8, name=f"rhs1_{u}")
                        for u in range(1, 5)}
                rhs2 = {k: sim.tile([128, KT, NB], F8, name=f"rhs2_{k}")
                        for k in range(1, 8)}

                cc0r = cc_out[0][:].rearrange(
                    "(g p) (a m) -> g p a m", p=128, a=KT)
                cc1r = cc_out[1][:].rearrange(
                    "(g p) (a m) -> g p a m", p=128, a=KT)
                for u in range(1, 5):
                    nc.sync.dma_start(rhs1[u][:], cc0r[g_n1[u]])
                for k in range(1, 8):
                    gk = g_n1[k] if k <= 3 else g_n2[k]
                    nc.sync.dma_start(rhs2[k][:], cc1r[gk])

                # pending colsum pass emitters, delayed by one unit
                pending = []

                def emit_pending():
                    while pending:
                        pending.pop(0)()

                def sim_unit(lhs, rhs_fn, bias, parts, pslot, colsum_dst,
                             keep_es, act_accum=False):
                    """One [NB x NB] unit: 8 row tiles x (4 DR MMs + exp ACT).
                    rhs_fn(kt2, c0, c1) -> AP of [128, c1-c0, 2] moving pairs.
                    colsum_dst: None or a dram AP receiving [1, NB]."""
                    es_tiles = []
                    es_pair = None
                    for mt in range(MT):
                        ps = sp.tile([128, NB], F32, tag="ps_sim")
                        for kt2 in range(KT2):
                            for ck in range(2):
                                nc.tensor.matmul(
                                    ps[:, ck * NCHUNK:(ck + 1) * NCHUNK],
                                    lhsT=lhs[:, 2 * kt2:2 * kt2 + 2,
                                             mt * 128:(mt + 1) * 128],
                                    rhs=rhs_fn(kt2, ck * NCHUNK,
                                               (ck + 1) * NCHUNK),
                                    start=(kt2 == 0), stop=(kt2 == KT2 - 1),
                                    perf_mode=DR)
                        if mt % 2 == 0:
                            es_pair = esp.tile([128, 2, NB], F8, tag="es")
                            if keep_es:
                                es_tiles.append(es_pair)
                        es = es_pair[:, mt % 2, :]
                        # rowsums: 2 tiles/unit on ACT accum, rest on DVE
                        if act_accum or mt < 2:
                            nc.scalar.activation(
                                es, ps[:], AF.Exp,
                                scale=TAU_INV / (F8SCALE * F8SCALE), bias=bias,
                                accum_out=parts[:, mt, pslot:pslot + 1])
                        else:
                            nc.scalar.activation(
                                es, ps[:], AF.Exp,
                                scale=TAU_INV / (F8SCALE * F8SCALE), bias=bias)
                            nc.vector.tensor_reduce(
                                parts[:, mt, pslot:pslot + 1], es,
                                mybir.AxisListType.X, ALU.add)
                        # emit the previous unit's colsum pass midway
                        if mt == 3:
                            emit_pending()

                    if colsum_dst is not None:
                        def colsum_pass(es_tiles=es_tiles,
                                        colsum_dst=colsum_dst):
                            cs = csp.tile([1, NB], F32, tag="cs")
                            for ck in range(2):
                                psc = cp.tile([1, NCHUNK], F32, tag="psc")
                                for pr in range(MT // 2):
                                    nc.tensor.matmul(
                                        psc[:],
                                        lhsT=ones_dr[:, :, 0:1],
                                        rhs=es_tiles[pr][:, :,
                                                         ck * NCHUNK:
                                                         (ck + 1) * NCHUNK],
                                        start=(pr == 0),
                                        stop=(pr == MT // 2 - 1),
                                        perf_mode=DR,
                                        skip_group_check=True)
                                nc.vector.tensor_copy(
                                    cs[:, ck * NCHUNK:(ck + 1) * NCHUNK],
                                    psc[:])
                            nc.sync.dma_start(colsum_dst, cs[:])
                        pending.append(colsum_pass)

                def loc_rhs(v):
                    return lambda kt2, c0, c1: \
                        n_loc[v][:, 2 * kt2:2 * kt2 + 2, c0:c1]

                def rhs1_rhs(u):
                    return lambda kt2, c0, c1: \
                        rhs1[u][:, 2 * kt2:2 * kt2 + 2, c0:c1]

                def n2s_rhs(k):
                    return lambda kt2, c0, c1: \
                        rhs2[k][:, 2 * kt2:2 * kt2 + 2, c0:c1]

                n2d_rhs = n2s_rhs

                # --- diagonal units (only need n_loc; run under the AGs) ---
                sim_unit(n_loc[0], loc_rhs(0), 0.0, parts11, 0, None,
                         False, act_accum=True)
                sim_unit(n_loc[1], loc_rhs(1), 0.0, parts22, 0, None,
                         False, act_accum=True)
                # --- S12 diagonal block: rhs is the local n2 ---
                dst = rs_in_r[my_slot, 2, :].unsqueeze(0)
                sim_unit(n_loc[0], loc_rhs(1), 0.0, parts12, 0, dst, True)

                # --- S11 off-diagonal units u=1..4 (need AG1) ---
                for u in range(1, 5):
                    bias = negln2 if u == 4 else 0.0
                    dst = rs_in_r[slot_n1[u], 0, :].unsqueeze(0)
                    sim_unit(n_loc[0], rhs1_rhs(u), bias, parts11, u, dst,
                             True)

                # --- S12 off-diagonal units k=1..7 (need AG2) ---
                for k in range(1, 8):
                    dst = rs_in_r[slot_12[k], 2, :].unsqueeze(0)
                    sim_unit(n_loc[0], n2s_rhs(k), 0.0, parts12, k, dst, True)

                # --- S22 off-diagonal units u=4..7 (need AG2) ---
                for u in range(4, 8):
                    bias = negln2 if u == 4 else 0.0
                    dst = rs_in_r[slot_n2[u], 1, :].unsqueeze(0)
                    sim_unit(n_loc[1], n2d_rhs(u), bias, parts22,
                             u - 3, dst, True)

                emit_pending()

                # --- my-rows rowsum partials stay LOCAL (only colsums
                # need the cross-core reduce) ---
                my11 = sim.tile([128, MT], F32)
                my22 = sim.tile([128, MT], F32)
                rs12l = sim.tile([128, MT], F32)
                nc.vector.tensor_reduce(my11[:], parts11[:],
                                        mybir.AxisListType.X, ALU.add)
                nc.vector.tensor_reduce(my22[:], parts22[:],
                                        mybir.AxisListType.X, ALU.add)
                nc.vector.tensor_reduce(rs12l[:], parts12[:],
                                        mybir.AxisListType.X, ALU.add)

                if DEBUG:
                    for v in range(2):
                        nc.sync.dma_start(dbg_rn[v:v + 1, :], rn_vec[v][:])

                # dummy Ln: pulls the ln act-table load into the RS window
                dln = sim.tile([1, 1], F32)
                nc.scalar.activation(dln[:], ones_col[0:1, 0:1], AF.Ln)

                # --- ReduceScatter: slot c -> colsum contributions for my
                # rows (class 0: S11, 1: S22, 2: S21) ---
                nc.gpsimd.collective_compute(
                    "ReduceScatter", ALU.add, replica_groups=rg,
                    ins=[rs_in.opt()], outs=[rs_out.opt()])

                # --- final assembly ---
                # contiguous load [24, 128] then transpose via identity matmul
                rsC = sim.tile([24, 128], F32)
                nc.sync.dma_start(
                    rsC[:], rs_out[:].rearrange("c (mt p) -> (c mt) p", p=128))
                d12 = sim.tile([128, 2 * MT], F32)
                d1 = d12[:, 0:MT]
                d2 = d12[:, MT:2 * MT]
                with tc.tile_pool(name="fin_psum", bufs=1,
                                  space="PSUM") as fp:
                    ptr = fp.tile([128, 24], F32)
                    nc.tensor.matmul(ptr[:], lhsT=rsC[:],
                                     rhs=id_sb[0:24, 0:24],
                                     start=True, stop=True)
                    # totals (for debug + denominators)
                    t0 = sim.tile([128, MT], F32)
                    t1d = sim.tile([128, MT], F32)
                    nc.vector.tensor_tensor(t0[:], ptr[:, 0:MT], my11[:],
                                            ALU.add)
                    nc.vector.tensor_tensor(t1d[:], ptr[:, MT:2 * MT],
                                            my22[:], ALU.add)
                    nc.vector.tensor_tensor(d1, t0[:], rs12l[:], ALU.add)
                    nc.vector.tensor_tensor(d2, t1d[:],
                                            ptr[:, 2 * MT:3 * MT], ALU.add)
                    nc.vector.tensor_scalar_add(d12[:], d12[:], -E2)
                    if DEBUG:
                        t2 = sim.tile([128, MT], F32)
                        nc.vector.tensor_copy(t2[:], ptr[:, 2 * MT:3 * MT])
                        nc.sync.dma_start(dbg_rs[:, 0:MT], t0[:])
                        nc.sync.dma_start(dbg_rs[:, MT:2 * MT], t1d[:])
                        nc.sync.dma_start(dbg_rs[:, 2 * MT:3 * MT], t2[:])
                        nc.sync.dma_start(dbg_d[:, 0:2 * MT], d12[:])
                        nc.sync.dma_start(dbg_d[:, 2 * MT:3 * MT], rs12l[:])
                        nc.sync.dma_start(dbg_d[:, 3 * MT:4 * MT], my11[:])
                    nc.scalar.activation(d12[:], d12[:], AF.Ln)
                    lsum = sim.tile([128, MT], F32)
                    nc.vector.tensor_tensor(lsum[:], d1, d2, ALU.add)
                    lrow = sim.tile([128, 1], F32)
                    nc.vector.tensor_reduce(lrow[:], lsum[:],
                                            mybir.AxisListType.X, ALU.add)
                    pfin = fp.tile([1, 1], F32)
                    nc.tensor.matmul(pfin[:], lhsT=ones_col[:], rhs=lrow[:],
                                     start=True, stop=True)
                    fin = sim.tile([1, 1], F32)
                    nc.vector.tensor_scalar_mul(fin[:], pfin[:], 0.5)
                    p2 = sim.tile([1, 1], F32)
                    nc.vector.tensor_scalar_mul(p2[:], pos_sum[:], 2.0)
                    nc.vector.tensor_tensor(fin[:], fin[:], p2[:],
                                            ALU.subtract)
                    nc.sync.dma_start(out, fin[:])

            sp_pool_cm.__exit__(None, None, None)
            es_pool_cm.__exit__(None, None, None)

    nc.compile()
    return nc


def _prep_inputs(z1, z2, fc1_w, fc1_b, fc2_w, fc2_b):
    """Host-side shard + layout prep. Returns in_maps for the 8 cores."""
    w1t = np.ascontiguousarray(fc1_w.T).reshape(KT, 128, D).transpose(1, 0, 2)
    w1t = np.ascontiguousarray(w1t, dtype=np.float16)
    w2t = np.ascontiguousarray(fc2_w.T).reshape(KT, 128, D).transpose(1, 0, 2)
    w2t = np.ascontiguousarray(w2t, dtype=np.float16)
    b1 = np.ascontiguousarray(fc1_b.reshape(KT, 128).T, dtype=np.float32)
    b2 = np.ascontiguousarray(fc2_b.reshape(KT, 128).T, dtype=np.float32)

    in_maps = []
    for c in range(N_CORES):
        blk1 = z1[c * NB:(c + 1) * NB].T            # [512, 1024]
        blk2 = z2[c * NB:(c + 1) * NB].T
        zt1 = np.ascontiguousarray(
            blk1.reshape(KT, 128, NB).transpose(1, 0, 2), dtype=np.float16)
        zt2 = np.ascontiguousarray(
            blk2.reshape(KT, 128, NB).transpose(1, 0, 2), dtype=np.float16)
        gidx = np.zeros((3, 8), np.uint32)
        for u in range(1, 5):
            gidx[0, u] = (c + u) % N_CORES
        for u in range(4, 8):
            gidx[1, u] = (c + u) % N_CORES
        gidx[2, 0] = c
        in_maps.append({"zt1": zt1, "zt2": zt2, "w1t": w1t, "w2t": w2t,
                        "b1": b1, "b2": b2, "gidx": gidx,
                        "ident": np.eye(128, dtype=np.float32)})
    return in_maps


def kernel(z1, z2, fc1_w, fc1_b, fc2_w, fc2_b):
    global LAST_EXEC_NS
    z1 = np.asarray(z1, dtype=np.float32)
    z2 = np.asarray(z2, dtype=np.float32)
    fc1_w = np.asarray(fc1_w, dtype=np.float32)
    fc1_b = np.asarray(fc1_b, dtype=np.float32)
    fc2_w = np.asarray(fc2_w, dtype=np.float32)
    fc2_b = np.asarray(fc2_b, dtype=np.float32)

    if "nc" not in _CACHE:
        _CACHE["nc"] = _build_program()
    nc = _CACHE["nc"]

    in_maps = _prep_inputs(z1, z2, fc1_w, fc1_b, fc2_w, fc2_b)
    res = run_bass_kernel_spmd(nc, in_maps, core_ids=list(range(N_CORES)),
                               trace=TRACE)
    LAST_EXEC_NS = res.exec_time_ns
    total = math.fsum(float(r["out"][0, 0]) for r in res.results)
    return np.float32(total / N)
